# revision 55
# baseline (speedup 1.0000x reference)
"""Trainium2 Bass kernel for nn_Block_11897059410591 (MLA transformer block).

Sharding over 8 NeuronCores: core c = (batch b=c//2, head-half h0=(c%2)*8).
Each core computes LN1/kvd/kvu/RoPE for its whole batch, causal attention for
its 8 heads, a partial output projection (contracted over its heads) that is
pair-AllReduced, then the FFN with d_ff split in half across the pair and a
second pair-AllReduce. Both cores of a pair end with the identical full-batch
output; the host keeps the even core's copy.

proj and f2 emit token-major partials so the collective payloads are
[tokens, C] and the residual/LN2/final paths need no PE transposes; the final
output recomputes x + sa + ff from DRAM so nothing is buffered across the
back half. The trailing f2 collective is split 256/128/128 so the exposed
tail is one 0.25 MB AllReduce.
"""
import sys

if "/opt/trn_rl_repo" not in sys.path:
    sys.path.insert(0, "/opt/trn_rl_repo")

import numpy as np
import ml_dtypes


def _ensure_ntff_hook():
    """antenv.axon_hooks is missing in this image; shim it so
    run_bass_kernel_spmd(trace=True) can capture NTFF profiles."""
    try:
        from antenv import axon_hooks  # noqa: F401
        return
    except ImportError:
        pass
    try:
        import types
        import importlib.util
        m = types.ModuleType("antenv.axon_hooks")
        _hook = [None]
        m.set_axon_ntff_profile_hook = lambda h: _hook.__setitem__(0, h)
        m.get_axon_ntff_profile_hook = lambda: _hook[0]
        sys.modules["antenv.axon_hooks"] = m
        import antenv
        antenv.axon_hooks = m
        spec = importlib.util.spec_from_file_location(
            "_trn_boot_shim", "/root/.axon_site/trn_agent_boot/trn_boot.py")
        tb = importlib.util.module_from_spec(spec)
        spec.loader.exec_module(tb)
        hook = tb._ntff_profile_via_ctypes("/opt/axon/libaxon_pjrt.so")
        m.set_axon_ntff_profile_hook(hook)
    except Exception as e:  # degrade to trace-less operation
        print(f"ntff hook shim failed ({e}); tracing disabled", file=sys.stderr)


_ensure_ntff_hook()

import concourse.bass as bass
import concourse.mybir as mybir
import concourse.tile as tile
from concourse import bacc
from concourse.bass_utils import run_bass_kernel_spmd
from concourse.masks import make_identity

F32 = mybir.dt.float32
BF = mybir.dt.bfloat16
BF16 = ml_dtypes.bfloat16
AF = mybir.ActivationFunctionType
ALU = mybir.AluOpType

B, T, C = 4, 2048, 1024
H, D, R, FF = 16, 64, 512, 4096
HL = 8              # heads per core
HD = HL * D         # 512
FH = FF // 2        # 2048, d_ff half per core
P = 128
NT = T // P         # 16 token chunks
NCC = C // P        # 8 C chunks
NRC = R // P        # 4 R chunks
NSL = 4             # token slabs for the back half
SLB = T // NSL      # 512 tokens per slab
LN_EPS = 1e-5

TRACE = False
_CACHE = {}


def _rope_tables():
    inv_freq = 1.0 / (10000.0 ** (np.arange(0, D, 2, dtype=np.float32) / D))
    t = np.arange(T, dtype=np.float32)
    freqs = np.outer(t, inv_freq)
    emb = np.concatenate([freqs, freqs], axis=-1)  # [T, D]
    cos = np.cos(emb).astype(np.float32)
    sin = np.sin(emb).astype(np.float32)
    sinf = sin.copy()
    sinf[:, : D // 2] = -sinf[:, : D // 2]
    return cos, sinf


def _build(flags):
    (ln1_triv, kvln_triv, ln2_triv, pb0, f1b0, f2b0) = flags
    nc = bacc.Bacc("TRN2", target_bir_lowering=False, debug=False,
                   enable_asserts=False, num_devices=8)

    x_d = nc.dram_tensor("x_loc", [T, C], F32, kind="ExternalInput").ap()
    qw_d = nc.dram_tensor("qw_loc", [C, HD], BF, kind="ExternalInput").ap()
    kvd_d = nc.dram_tensor("kvd_w", [C, R], BF, kind="ExternalInput").ap()
    kvuk_d = nc.dram_tensor("kvu_k", [R, HD], BF, kind="ExternalInput").ap()
    kvuv_d = nc.dram_tensor("kvu_v", [R, HD], BF, kind="ExternalInput").ap()
    pw_d = nc.dram_tensor("proj_w_loc", [HD, C], BF, kind="ExternalInput").ap()
    f1_d = nc.dram_tensor("f1_w_loc", [C, FH], BF, kind="ExternalInput").ap()
    f2_d = nc.dram_tensor("f2_w_loc", [FH, C], BF, kind="ExternalInput").ap()
    cos_d = nc.dram_tensor("cos_t", [T, D], BF, kind="ExternalInput").ap()
    sinf_d = nc.dram_tensor("sinf_t", [T, D], BF, kind="ExternalInput").ap()
    out_d = nc.dram_tensor("out_loc", [T, C], F32, kind="ExternalOutput").ap()

    opt_ins = {}
    if not ln1_triv:
        opt_ins["ln1_w"] = nc.dram_tensor("ln1_w", [C], F32, kind="ExternalInput").ap()
        opt_ins["ln1_b"] = nc.dram_tensor("ln1_b", [C], F32, kind="ExternalInput").ap()
    if not kvln_triv:
        opt_ins["kvln_w"] = nc.dram_tensor("kvln_w", [R], F32, kind="ExternalInput").ap()
        opt_ins["kvln_b"] = nc.dram_tensor("kvln_b", [R], F32, kind="ExternalInput").ap()
    if not ln2_triv:
        opt_ins["ln2_w"] = nc.dram_tensor("ln2_w", [C], F32, kind="ExternalInput").ap()
        opt_ins["ln2_b"] = nc.dram_tensor("ln2_b", [C], F32, kind="ExternalInput").ap()
    if not pb0:
        opt_ins["proj_b"] = nc.dram_tensor("proj_b", [C], F32, kind="ExternalInput").ap()
    if not f1b0:
        opt_ins["f1_b"] = nc.dram_tensor("f1_b_loc", [FH], F32, kind="ExternalInput").ap()
    if not f2b0:
        opt_ins["f2_b"] = nc.dram_tensor("f2_b", [C], F32, kind="ExternalInput").ap()

    # internal DRAM (all collective payloads token-major)
    cc1_in = [nc.dram_tensor(f"cc1_in{n}", [SLB, C], BF).ap()
              for n in range(NSL)]
    cc1_out = [nc.dram_tensor(f"cc1_out{n}", [SLB, C], BF).ap()
               for n in range(NSL)]
    ccw_in = nc.dram_tensor("ccw_in", [P, 2], F32).ap()
    ccw_out = nc.dram_tensor("ccw_out", [P, 2], F32).ap()
    # f2 collective chunks: (token_start, n_tokens); only the last chunk is
    # small (per-op fixed cost ~10us dominates below ~0.5MB, so an evenly
    # fine split would serialize on the CC stream)
    CC2 = [(0, 512), (512, 512), (1024, 512), (1536, 384), (1920, 128)]
    cc2_in = [nc.dram_tensor(f"cc2_in{ci}", [ln, C], BF).ap()
              for ci, (t0, ln) in enumerate(CC2)]
    cc2_out = [nc.dram_tensor(f"cc2_out{ci}", [ln, C], BF).ap()
               for ci, (t0, ln) in enumerate(CC2)]
    groups = [[0, 1], [2, 3], [4, 5], [6, 7]]

    def bcast_free(ap2d, n, width):
        """[P, width] AP -> [P, n, width] with 0-step middle dim."""
        return bass.AP(tensor=ap2d.tensor, offset=ap2d.offset,
                       ap=[ap2d.ap[0], [0, n], [1, width]])

    def ln_stats(pool, src_ap, width, eps_t):
        """Per-partition (mean, rstd) of src_ap [P, width]."""
        ngr = (width + 511) // 512
        st6 = pool.tile([P, ngr, 6], F32, tag="st6")
        sv = src_ap.rearrange("p (g d) -> p g d", g=ngr)
        for g in range(ngr):
            nc.vector.bn_stats(out=st6[:, g, :], in_=sv[:, g, :])
        mv = pool.tile([P, 2], F32, tag="mv")
        nc.vector.bn_aggr(out=mv, in_=st6)
        nc.scalar.activation(out=mv[:, 1:2], in_=mv[:, 1:2], func=AF.Sqrt,
                             bias=eps_t, scale=1.0)
        nc.vector.reciprocal(out=mv[:, 1:2], in_=mv[:, 1:2])
        return mv

    from contextlib import ExitStack
    with tile.TileContext(nc) as tc:
        with ExitStack() as ctx:
            const = ctx.enter_context(tc.tile_pool(name="const", bufs=1))
            ident = const.tile([P, P], BF)
            make_identity(nc, ident)
            eps_t = const.tile([P, 1], F32)
            nc.vector.memset(eps_t, LN_EPS)
            # S^T diagonal causal mask: keep (0) where col(q) >= row(k)
            maskT = const.tile([P, P], F32)
            nc.gpsimd.memset(maskT, 0.0)
            nc.gpsimd.affine_select(out=maskT, in_=maskT, compare_op=ALU.is_ge,
                                    fill=-1e9, base=0, pattern=[[1, P]],
                                    channel_multiplier=-1)
            mask_full = const.tile([P, P], F32)
            nc.vector.memset(mask_full, -1e9)
            # Written at the end of attention; back-half tiles that depend on
            # collective outputs take a 1-element probe-write reading this
            # first, so the scheduler cannot hoist their loads (and the DVE
            # ops behind them) into the attention region of the engine
            # queues, where a slow AllReduce would head-of-line-block it.
            gate_t = const.tile([1, 4], F32, name="gate")
            # [full | diag] combined mask for the kb==q0+1 block pair
            mask2 = const.tile([P, 2, P], F32)
            nc.vector.memset(mask2[:, 0, :], -1e9)
            nc.vector.tensor_copy(out=mask2[:, 1, :], in_=maskT)

            # CC-stream warm-up: a tiny AllReduce issued at t=0 absorbs the
            # collective stack's cold-start cost under the prep phase, so
            # the first real AllReduce runs at steady-state speed.
            ccw_t = const.tile([P, 2], F32)
            nc.vector.memset(ccw_t, 1.0)
            nc.sync.dma_start(out=ccw_in, in_=ccw_t)
            nc.gpsimd.collective_compute(
                "AllReduce", ALU.add, replica_groups=groups,
                ins=[ccw_in], outs=[ccw_out])

            def dram_row_bcast(name, ap1d, width):
                t = const.tile([P, width], F32, name=name)
                src = bass.AP(tensor=ap1d.tensor, offset=ap1d.offset,
                              ap=[[0, P], [1, width]])
                nc.sync.dma_start(out=t, in_=src)
                return t

            ln1_wt = ln1_bt = ln2_wt = ln2_bt = kvln_wt = kvln_bt = None
            if not ln1_triv:
                ln1_wt = dram_row_bcast("ln1w_b", opt_ins["ln1_w"], C)
                ln1_bt = dram_row_bcast("ln1b_b", opt_ins["ln1_b"], C)
            if not kvln_triv:
                kvln_wt = dram_row_bcast("kvlnw_b", opt_ins["kvln_w"], R)
                kvln_bt = dram_row_bcast("kvlnb_b", opt_ins["kvln_b"], R)
            if not ln2_triv:
                ln2_wt = dram_row_bcast("ln2w_b", opt_ins["ln2_w"], C)
                ln2_bt = dram_row_bcast("ln2b_b", opt_ins["ln2_b"], C)
            projb_t = f1b_t = f2b_t = None
            if not pb0:
                # token-major proj output -> bias is a broadcast row [*, C]
                projb_t = dram_row_bcast("projb_b", opt_ins["proj_b"], C)
            if not f1b0:
                f1b_t = const.tile([P, FH // P], F32, name="f1b")
                nc.sync.dma_start(out=f1b_t, in_=opt_ins["f1_b"].rearrange(
                    "(m p) -> p m", p=P))
            if not f2b0:
                f2b_t = dram_row_bcast("f2b_b", opt_ins["f2_b"], C)

            def load_chunks(pool, dram_ap, nk, width, name):
                t = pool.tile([P, nk, width], BF, name=name)
                for k in range(nk):
                    nc.sync.dma_start(out=t[:, k, :],
                                      in_=dram_ap[k * P:(k + 1) * P, :])
                return t

            # ---- long-lived pools (created early; closed last, LIFO) ------
            pool_xs = ctx.enter_context(tc.tile_pool(name="xs2", bufs=3))
            pool_h2T = ctx.enter_context(tc.tile_pool(name="h2T", bufs=2))
            h2T_sl = [None] * NSL
            fivb_pools = {}

            def emit_5b(n):
                """Residual + LN2 + h2T for token slab n. cc1_out is
                token-major so no PE transposes are needed before LN2; only
                h2 -> h2T is transposed (for the f1 matmul rhs), in a second
                pass so the PE doesn't stall on the DVE LN chain."""
                h2T = pool_h2T.tile([P, NCC, SLB], BF, tag="h2T")
                h2T_sl[n] = h2T
                sa_in = fivb_pools["sa_in"].tile([P, SLB // P, C], BF,
                                                 tag="sa_in", name="sa_in")
                # probe-write: orders the loads after attention (see gate_t)
                nc.vector.tensor_copy(out=sa_in[0:1, 0, 0:4], in_=gate_t)
                for itl in range(SLB // P):
                    nc.sync.dma_start(
                        out=sa_in[:, itl, :],
                        in_=cc1_out[n][itl * P:(itl + 1) * P, :])
                h2s = []
                for itl in range(SLB // P):
                    it = n * (SLB // P) + itl
                    sl = slice(it * P, (it + 1) * P)
                    xt = pool_xs.tile([P, C], F32, tag="xt2")
                    nc.sync.dma_start(out=xt, in_=x_d[sl, :])
                    x2t = fivb_pools["x2"].tile([P, C], F32, tag="x2t",
                                                name="x2t")
                    nc.vector.tensor_add(x2t, xt, sa_in[:, itl, :])
                    mv = ln_stats(fivb_pools["st2"], x2t, C, eps_t)
                    h2 = fivb_pools["tok2"].tile([P, C], BF, tag="h2", name="h2")
                    nc.vector.tensor_scalar(out=h2, in0=x2t,
                                            scalar1=mv[:, 0:1],
                                            scalar2=mv[:, 1:2],
                                            op0=ALU.subtract, op1=ALU.mult)
                    if ln2_wt is not None:
                        nc.vector.tensor_mul(h2, h2, ln2_wt)
                        nc.vector.tensor_add(h2, h2, ln2_bt)
                    h2s.append((h2, slice(itl * P, (itl + 1) * P)))
                for h2, lsl in h2s:
                    pt2 = ps_tr1.tile([P, C], BF, tag="p1k")
                    for kc in range(NCC):
                        nc.tensor.transpose(pt2[:, kc * P:(kc + 1) * P],
                                            h2[:, kc * P:(kc + 1) * P],
                                            ident)
                    nc.scalar.copy(
                        out=h2T[:, :, lsl],
                        in_=pt2.rearrange("p (kc t) -> p kc t", kc=NCC))

            # ---------------- qkv/att scope --------------------------------
            mid = ExitStack()
            pool_qkv = mid.enter_context(tc.tile_pool(name="qkv", bufs=1))
            pool_att = mid.enter_context(tc.tile_pool(name="att", bufs=2))
            qT = pool_qkv.tile([P, HL // 2, T], BF)   # [(2h,64d), hp, T]
            kT = pool_qkv.tile([P, HL // 2, T], BF)
            vaug = pool_qkv.tile([P, NT, HL, D + 1], BF)

            # ------- prep (fused per-chunk): LN1 -> q/kv/RoPE --------------
            # One loop over 16 token chunks keeps the PE fed with matmuls
            # from the start (HAM stays warm) and lets DVE LN/RoPE for chunk
            # i+1 overlap PE work for chunk i. hT/ckvT chunks are only used
            # within their iteration, so they rotate in small pools.
            with ExitStack() as prep:
                pool_hT = prep.enter_context(tc.tile_pool(name="hT", bufs=3))
                pool_ckvT = prep.enter_context(tc.tile_pool(name="ckvT", bufs=3))
                pool_xsp = prep.enter_context(tc.tile_pool(name="xs", bufs=4))
                pool_stp = prep.enter_context(tc.tile_pool(name="st", bufs=6))
                pool_tokp = prep.enter_context(tc.tile_pool(name="tok", bufs=4))
                pool_w = prep.enter_context(tc.tile_pool(name="wts", bufs=1))
                pool_cs = prep.enter_context(tc.tile_pool(name="cs", bufs=4))
                pool_ro = prep.enter_context(tc.tile_pool(name="ro", bufs=6))
                # 4 accumulators per chunk: bufs=6 gives 1.5 chunks of
                # cross-chunk pipeline depth (the 3-slot ring serialized
                # chunks almost completely)
                ps_big = prep.enter_context(
                    tc.tile_pool(name="psbig", bufs=6, space="PSUM"))
                ps_tr = prep.enter_context(
                    tc.tile_pool(name="pstr", bufs=2, space="PSUM"))

                # x/cos loads for the first chunks are emitted BEFORE the
                # weight loads so the LN1 chain starts at ~2us instead of
                # queueing behind 4MB of weights on the same DMA queue
                xstash = {}

                def load_x(it):
                    sl = slice(it * P, (it + 1) * P)
                    xt = pool_xsp.tile([P, C], F32)
                    nc.sync.dma_start(out=xt, in_=x_d[sl, :])
                    cos_sb = pool_cs.tile([P, D], BF, tag="cos")
                    nc.sync.dma_start(out=cos_sb, in_=cos_d[sl, :])
                    sinf_sb = pool_cs.tile([P, D], BF, tag="sinf")
                    nc.sync.dma_start(out=sinf_sb, in_=sinf_d[sl, :])
                    xstash[it] = (xt, cos_sb, sinf_sb)

                load_x(0)
                load_x(1)
                kvdw_sb = load_chunks(pool_w, kvd_d, NCC, R, "kvdw")
                qw_sb = load_chunks(pool_w, qw_d, NCC, HD, "qw")
                kvuk_sb = load_chunks(pool_w, kvuk_d, NRC, HD, "kvuk")
                kvuv_sb = load_chunks(pool_w, kvuv_d, NRC, HD, "kvuv")

                def rope(ps, cos_sb, sinf_sb):
                    # ACT evicts psum to a flat bf16 tile (plain write, no
                    # view); all-bf16 DVE ops then run at 2x rate
                    qbf = pool_ro.tile([P, HD], BF, tag="qbf", name="qbf")
                    nc.scalar.copy(out=qbf, in_=ps)
                    psv = qbf.rearrange("p (h d) -> p h d", d=D)
                    t1 = pool_ro.tile([P, HL, D], BF, tag="t1")
                    nc.vector.tensor_mul(t1, psv, bcast_free(cos_sb, HL, D))
                    t2 = pool_ro.tile([P, HL, D], BF, tag="t2")
                    half = D // 2
                    sfv = sinf_sb
                    nc.vector.tensor_mul(
                        t2[:, :, 0:half],
                        bass.AP(tensor=psv.tensor, offset=psv.offset + half,
                                ap=[psv.ap[0], [D, HL], [1, half]]),
                        bass.AP(tensor=sfv.tensor, offset=sfv.offset,
                                ap=[sfv.ap[0], [0, HL], [1, half]]))
                    nc.vector.tensor_mul(
                        t2[:, :, half:D],
                        bass.AP(tensor=psv.tensor, offset=psv.offset,
                                ap=[psv.ap[0], [D, HL], [1, half]]),
                        bass.AP(tensor=sfv.tensor, offset=sfv.offset + half,
                                ap=[sfv.ap[0], [0, HL], [1, half]]))
                    ro = pool_ro.tile([P, HL, D], BF, tag="ro")
                    nc.vector.tensor_add(ro, t1, t2)
                    return ro.rearrange("p h d -> p (h d)")

                def evict_pairs(ro_flat, dstT, sl):
                    # [tok, (2 heads x 64d)] pair-chunks -> dstT[(par,d), hp]
                    for hp in range(HL // 2):
                        pt = ps_tr.tile([P, P], BF, tag="ptr")
                        nc.tensor.transpose(
                            pt, ro_flat[:, hp * P:(hp + 1) * P], ident)
                        if hp % 2 == 0:
                            nc.scalar.copy(out=dstT[:, hp, sl], in_=pt)
                        else:
                            nc.vector.tensor_copy(out=dstT[:, hp, sl], in_=pt)

                # Explicit two-stage software pipeline: stage A of chunk
                # i+1 is emitted before stage B of chunk i, so every engine
                # queue interleaves adjacent chunks (the scheduler's cost
                # model underestimates DVE latency and otherwise serializes
                # the whole per-chunk chain).
                stash = {}

                def stage_a(it):
                    sl = slice(it * P, (it + 1) * P)
                    if it not in xstash:
                        load_x(it)
                    xt, cos_sb, sinf_sb = xstash[it]

                    mv = ln_stats(pool_stp, xt, C, eps_t)
                    ht = pool_tokp.tile([P, C], BF, tag="ht")
                    nc.vector.tensor_scalar(out=ht, in0=xt,
                                            scalar1=mv[:, 0:1], scalar2=mv[:, 1:2],
                                            op0=ALU.subtract, op1=ALU.mult)
                    if ln1_wt is not None:
                        nc.vector.tensor_mul(ht, ht, ln1_wt)
                        nc.vector.tensor_add(ht, ht, ln1_bt)
                    hTc = pool_hT.tile([P, NCC, P], BF, tag="hTc")
                    for kc in range(NCC):
                        pt = ps_tr.tile([P, P], BF, tag="ptr")
                        nc.tensor.transpose(pt, ht[:, kc * P:(kc + 1) * P], ident)
                        if kc % 2 == 0:
                            nc.scalar.copy(out=hTc[:, kc, :], in_=pt)
                        else:
                            nc.vector.tensor_copy(out=hTc[:, kc, :], in_=pt)

                    psq = ps_big.tile([P, HD], F32, tag="psb")
                    for kc in range(NCC):
                        nc.tensor.matmul(psq, lhsT=hTc[:, kc, :],
                                         rhs=qw_sb[:, kc, :],
                                         start=(kc == 0), stop=(kc == NCC - 1))
                    ps = ps_big.tile([P, R], F32, tag="psb")
                    for kc in range(NCC):
                        nc.tensor.matmul(ps, lhsT=hTc[:, kc, :],
                                         rhs=kvdw_sb[:, kc, :],
                                         start=(kc == 0), stop=(kc == NCC - 1))
                    stash[it] = (psq, ps, cos_sb, sinf_sb)

                def stage_b(it):
                    sl = slice(it * P, (it + 1) * P)
                    psq, ps, cos_sb, sinf_sb = stash.pop(it)
                    ro_q = rope(psq, cos_sb, sinf_sb)

                    mv = ln_stats(pool_stp, ps, R, eps_t)
                    ct = pool_tokp.tile([P, R], BF, tag="ckvtok")
                    nc.vector.tensor_scalar(out=ct, in0=ps,
                                            scalar1=mv[:, 0:1], scalar2=mv[:, 1:2],
                                            op0=ALU.subtract, op1=ALU.mult)
                    if kvln_wt is not None:
                        nc.vector.tensor_mul(ct, ct, kvln_wt)
                        nc.vector.tensor_add(ct, ct, kvln_bt)

                    evict_pairs(ro_q, qT, sl)

                    ckvTc = pool_ckvT.tile([P, NRC, P], BF, tag="ckvTc")
                    for rc in range(NRC):
                        pt = ps_tr.tile([P, P], BF, tag="ptr")
                        nc.tensor.transpose(pt, ct[:, rc * P:(rc + 1) * P], ident)
                        if rc % 2 == 0:
                            nc.scalar.copy(out=ckvTc[:, rc, :], in_=pt)
                        else:
                            nc.vector.tensor_copy(out=ckvTc[:, rc, :], in_=pt)

                    psk = ps_big.tile([P, HD], F32, tag="psb")
                    for rc in range(NRC):
                        nc.tensor.matmul(psk, lhsT=ckvTc[:, rc, :],
                                         rhs=kvuk_sb[:, rc, :],
                                         start=(rc == 0), stop=(rc == NRC - 1))
                    ro_k = rope(psk, cos_sb, sinf_sb)

                    psv = ps_big.tile([P, HD], F32, tag="psb")
                    for rc in range(NRC):
                        nc.tensor.matmul(psv, lhsT=ckvTc[:, rc, :],
                                         rhs=kvuv_sb[:, rc, :],
                                         start=(rc == 0), stop=(rc == NRC - 1))

                    evict_pairs(ro_k, kT, sl)

                    nc.vector.memset(vaug[:, it, :, D:D + 1], 1.0)
                    nc.scalar.copy(out=vaug[:, it, :, 0:D],
                                   in_=psv.rearrange("p (h d) -> p h d", d=D))

                stage_a(0)
                for it in range(NT):
                    if it + 1 < NT:
                        stage_a(it + 1)
                    stage_b(it)

            # ---- Phase 3+5a+5b: attention / proj / residual interleaved ---
            with ExitStack() as attn:
                pool_p = attn.enter_context(tc.tile_pool(name="pexp", bufs=8))
                pool_s = attn.enter_context(tc.tile_pool(name="srow", bufs=4))
                pool_bc = attn.enter_context(tc.tile_pool(name="bc", bufs=4))
                pool_pw = attn.enter_context(tc.tile_pool(name="pw", bufs=1))
                pool_sa = attn.enter_context(tc.tile_pool(name="sa", bufs=3))
                ps_s = attn.enter_context(
                    tc.tile_pool(name="pss", bufs=2, space="PSUM"))
                ps_o = attn.enter_context(
                    tc.tile_pool(name="pso", bufs=1, space="PSUM"))
                ps_pj = attn.enter_context(
                    tc.tile_pool(name="pspj", bufs=1, space="PSUM"))
                pw_sb = load_chunks(pool_pw, pw_d, NRC, C, "pw")
                NPAIR = NT // 2
                att_cur = [None]  # per-slab [(2h,64d), hp, SLB] tile

                def proj_slab(n):
                    # token-major partial: out[tok, C] = att[:, hp, tok].T @ pw
                    att = att_cur[0]
                    for itl in range(SLB // P):
                        tsl = slice(itl * P, (itl + 1) * P)
                        ps = ps_pj.tile([P, C], F32, tag="pspj")
                        for half in range(2):
                            hsl = slice(half * 512, (half + 1) * 512)
                            for hp in range(NRC):
                                nc.tensor.matmul(ps[:, hsl],
                                                 lhsT=att[:, hp, tsl],
                                                 rhs=pw_sb[:, hp, hsl],
                                                 start=(hp == 0),
                                                 stop=(hp == NRC - 1))
                        sa_t = pool_sa.tile([P, C], BF, tag="sat")
                        if projb_t is not None:
                            nc.vector.tensor_add(sa_t, ps, projb_t)
                        else:
                            nc.scalar.copy(out=sa_t, in_=ps)
                        nc.sync.dma_start(out=cc1_in[n][tsl, :], in_=sa_t)
                    nc.gpsimd.collective_compute(
                        "AllReduce", ALU.add, replica_groups=groups,
                        ins=[cc1_in[n]], outs=[cc1_out[n]])

                # Per head-pair, interleave the two heads (PE row groups 0-63
                # and 64-127) and pipeline qk of group g+1 ahead of av of
                # group g so the Exp never stalls the PE.
                GRP = 4  # key-blocks per score group
                for pi in range(NPAIR):
                    if pi % 2 == 0:
                        att_cur[0] = pool_att.tile([P, NRC, SLB], BF,
                                                   tag="attsl", name="attsl")
                    q0 = 2 * pi            # first q-block of pair
                    qsl = slice(q0 * P, (q0 + 2) * P)     # 256 queries
                    asl = slice((pi % 2) * 2 * P, (pi % 2 + 1) * 2 * P)
                    nkb = 2 * pi + 2
                    ngr = (nkb + GRP - 1) // GRP

                    for hp in range(HL // 2):
                        poA = ps_o.tile([P, 2 * P], F32, tag="poA", name="poA")
                        poB = ps_o.tile([P, 2 * P], F32, tag="poB", name="poB")
                        pos = {0: poA, 64: poB}

                        def qk_group(g, hr):
                            kbn = min(GRP, nkb - g * GRP)
                            pss = ps_s.tile([P, GRP * 2 * P], F32, tag="pss")
                            for j in range(kbn):
                                kb = g * GRP + j
                                jsl = slice(j * 2 * P, (j + 1) * 2 * P)
                                nc.tensor.matmul(
                                    pss[:, jsl],
                                    lhsT=kT[hr:hr + 64, hp, kb * P:(kb + 1) * P],
                                    rhs=qT[hr:hr + 64, hp, qsl],
                                    start=True, stop=True)
                                if kb == q0:
                                    nc.vector.tensor_add(
                                        pss[:, j * 2 * P:j * 2 * P + P],
                                        pss[:, j * 2 * P:j * 2 * P + P], maskT)
                                elif kb == q0 + 1:
                                    nc.vector.tensor_add(
                                        pss[:, jsl], pss[:, jsl],
                                        mask2.rearrange("p a b -> p (a b)"))
                            pexp = pool_p.tile([P, GRP * 2 * P], BF, tag="pexp")
                            nc.scalar.activation(out=pexp[:, 0:kbn * 2 * P],
                                                 in_=pss[:, 0:kbn * 2 * P],
                                                 func=AF.Exp,
                                                 scale=float(D) ** -0.5)
                            return pexp

                        def av_group(g, hr, pexp):
                            po = pos[hr]
                            h = 2 * hp + (1 if hr else 0)
                            kbn = min(GRP, nkb - g * GRP)
                            for j in range(kbn):
                                kb = g * GRP + j
                                nc.tensor.matmul(
                                    po[0:D + 1, :],
                                    lhsT=vaug[:, kb, h, :],
                                    rhs=pexp[:, j * 2 * P:(j + 1) * 2 * P],
                                    start=(kb == 0), stop=(kb == nkb - 1))

                        # all qk groups first (64-row array mode, the two
                        # heads in disjoint row groups), then all av groups
                        # (128-row mode): two mode switches per head-pair
                        # instead of two per group
                        pexps = []
                        for g in range(ngr):
                            pexps.append((qk_group(g, 0), qk_group(g, 64)))
                        for g in range(ngr):
                            av_group(g, 0, pexps[g][0])
                            av_group(g, 64, pexps[g][1])

                        # normalization fused into eviction
                        for hr in (0, 64):
                            po = pos[hr]
                            srow = pool_s.tile([1, 2 * P], F32, tag="srow")
                            nc.vector.tensor_copy(out=srow, in_=po[D:D + 1, :])
                            bc = pool_bc.tile([64, 2 * P], F32, tag="bc")
                            nc.gpsimd.partition_broadcast(out_ap=bc, in_ap=srow)
                            rc_ = pool_bc.tile([64, 2 * P], F32, tag="rc")
                            nc.vector.reciprocal_approx_fast(out=rc_, in_=bc)
                            nc.vector.tensor_mul(
                                att_cur[0][hr:hr + 64, hp, asl],
                                po[0:D, :], rc_)
                    if pi % 2 == 1:
                        proj_slab(pi // 2)

                # gate: depends on the last pair's attention eviction
                nc.vector.tensor_copy(out=gate_t,
                                      in_=att_cur[0][0:1, NRC - 1,
                                                     SLB - 4:SLB])

            mid.close()  # release qT/kT/vaug/att SBUF

            # ------- Back half: FFN + second AllReduce + final -------------
            with ExitStack() as bh:
                pool_fw = bh.enter_context(tc.tile_pool(name="fw", bufs=1))
                pool_g = bh.enter_context(tc.tile_pool(name="gT", bufs=2))
                pool_ev = bh.enter_context(tc.tile_pool(name="ev", bufs=3))
                pool_ffin = bh.enter_context(tc.tile_pool(name="ffin", bufs=3))
                pool_of = bh.enter_context(tc.tile_pool(name="of", bufs=3))
                ps_f = bh.enter_context(
                    tc.tile_pool(name="psf", bufs=2, space="PSUM"))
                ps_f2 = bh.enter_context(
                    tc.tile_pool(name="psf2", bufs=2, space="PSUM"))
                ps_tr1 = bh.enter_context(
                    tc.tile_pool(name="pstr1", bufs=2, space="PSUM"))
                fivb_pools["sa_in"] = bh.enter_context(
                    tc.tile_pool(name="sain", bufs=2))
                fivb_pools["x2"] = bh.enter_context(
                    tc.tile_pool(name="x2", bufs=2))
                fivb_pools["st2"] = bh.enter_context(
                    tc.tile_pool(name="st2", bufs=4))
                fivb_pools["tok2"] = bh.enter_context(
                    tc.tile_pool(name="tok2", bufs=4))

                # f1/f2 weight loads first: the DMAs overlap the attention
                # tail (fresh pool space, no released-zone wait on hot tiles)
                f1w_sb = load_chunks(pool_fw, f1_d, NCC, FH, "f1w")
                f2w_sb = load_chunks(pool_fw, f2_d, FH // P, C, "f2w")

                gT_sl = [None] * NSL
                NIT = SLB // P  # 4 token blocks per slab

                def f1_slab(n):
                    h2T = h2T_sl[n]
                    gT = pool_g.tile([P, FH // P, SLB], BF, tag="gT")
                    gT_sl[n] = gT
                    for m in range(FH // P):
                        ps = ps_f.tile([P, SLB], F32, tag="psf1")
                        for kc in range(NCC):
                            nc.tensor.matmul(ps,
                                             lhsT=f1w_sb[:, kc, m * P:(m + 1) * P],
                                             rhs=h2T[:, kc, :],
                                             start=(kc == 0),
                                             stop=(kc == NCC - 1))
                        if f1b_t is not None:
                            nc.scalar.activation(out=gT[:, m, :], in_=ps,
                                                 func=AF.Gelu,
                                                 bias=f1b_t[:, m:m + 1],
                                                 scale=1.0)
                        else:
                            nc.scalar.activation(out=gT[:, m, :], in_=ps,
                                                 func=AF.Gelu, scale=1.0)

                def cc2_home(it):
                    t = it * P
                    for ci, (t0, ln) in enumerate(CC2):
                        if t0 <= t < t0 + ln:
                            return ci, t - t0
                    raise AssertionError(it)

                def f2_block(it):
                    """f2 partial for global 128-token block it, token-major."""
                    n, itl = it // NIT, it % NIT
                    gT = gT_sl[n]
                    tsl = slice(itl * P, (itl + 1) * P)
                    ps = ps_f2.tile([P, C], F32, tag="psf2")
                    for half in range(2):
                        hsl = slice(half * 512, (half + 1) * 512)
                        for kf in range(FH // P):
                            nc.tensor.matmul(ps[:, hsl],
                                             lhsT=gT[:, kf, tsl],
                                             rhs=f2w_sb[:, kf, hsl],
                                             start=(kf == 0),
                                             stop=(kf == FH // P - 1))
                    ev = pool_ev.tile([P, C], BF, tag="ffev")
                    if f2b_t is not None:
                        nc.vector.tensor_add(ev, ps, f2b_t)
                    else:
                        nc.scalar.copy(out=ev, in_=ps)
                    ci, row = cc2_home(it)
                    nc.sync.dma_start(out=cc2_in[ci][row:row + P, :], in_=ev)

                def cc2_launch(ci):
                    nc.gpsimd.collective_compute(
                        "AllReduce", ALU.add, replica_groups=groups,
                        ins=[cc2_in[ci]], outs=[cc2_out[ci]])

                def emit_final(ci):
                    """out = x + sa + ff for collective chunk ci (no PE)."""
                    t0, ln = CC2[ci]
                    for j in range(ln // P):
                        itg = t0 // P + j
                        sl = slice(itg * P, (itg + 1) * P)
                        xt = pool_xs.tile([P, C], F32, tag="xt2")
                        nc.sync.dma_start(out=xt, in_=x_d[sl, :])
                        saf = pool_ffin.tile([P, C], BF, tag="sfin")
                        nc.vector.tensor_copy(out=saf[0:1, 0:4], in_=gate_t)
                        nc.sync.dma_start(
                            out=saf,
                            in_=cc1_out[itg // NIT][(itg % NIT) * P:
                                                    (itg % NIT + 1) * P, :])
                        ff = pool_ffin.tile([P, C], BF, tag="ffin")
                        nc.vector.tensor_copy(out=ff[0:1, 0:4], in_=gate_t)
                        nc.sync.dma_start(out=ff,
                                          in_=cc2_out[ci][j * P:(j + 1) * P, :])
                        ot = pool_of.tile([P, C], F32, tag="of")
                        nc.vector.tensor_add(ot, xt, saf)
                        nc.gpsimd.tensor_add(ot, ot, ff)
                        nc.sync.dma_start(out=out_d[sl, :], in_=ot)

                emit_5b(0)
                f1_slab(0)
                emit_5b(1)
                for it in range(0, 4):
                    f2_block(it)
                cc2_launch(0)
                f1_slab(1)
                emit_5b(2)
                for it in range(4, 8):
                    f2_block(it)
                cc2_launch(1)
                emit_final(0)
                f1_slab(2)
                emit_5b(3)
                for it in range(8, 12):
                    f2_block(it)
                cc2_launch(2)
                emit_final(1)
                f1_slab(3)
                f2_block(12)
                f2_block(13)
                f2_block(14)
                cc2_launch(3)
                emit_final(2)
                f2_block(15)
                cc2_launch(4)
                emit_final(3)
                emit_final(4)

    nc.compile()
    return nc


def kernel(**inputs):
    x = np.asarray(inputs["x"], dtype=np.float32)
    q_w = np.asarray(inputs["q_w"], dtype=np.float32)
    kvd_w = np.asarray(inputs["kvd_w"], dtype=np.float32)
    kvu_w = np.asarray(inputs["kvu_w"], dtype=np.float32)
    proj_w = np.asarray(inputs["proj_w"], dtype=np.float32)
    f1_w = np.asarray(inputs["f1_w"], dtype=np.float32)
    f2_w = np.asarray(inputs["f2_w"], dtype=np.float32)
    ln1_w = np.asarray(inputs["ln1_w"], dtype=np.float32)
    ln1_b = np.asarray(inputs["ln1_b"], dtype=np.float32)
    ln2_w = np.asarray(inputs["ln2_w"], dtype=np.float32)
    ln2_b = np.asarray(inputs["ln2_b"], dtype=np.float32)
    kvln_w = np.asarray(inputs["kvln_w"], dtype=np.float32)
    kvln_b = np.asarray(inputs["kvln_b"], dtype=np.float32)
    proj_b = np.asarray(inputs["proj_b"], dtype=np.float32)
    f1_b = np.asarray(inputs["f1_b"], dtype=np.float32)
    f2_b = np.asarray(inputs["f2_b"], dtype=np.float32)

    flags = (bool(np.allclose(ln1_w, 1) and np.allclose(ln1_b, 0)),
             bool(np.allclose(kvln_w, 1) and np.allclose(kvln_b, 0)),
             bool(np.allclose(ln2_w, 1) and np.allclose(ln2_b, 0)),
             bool(np.allclose(proj_b, 0)),
             bool(np.allclose(f1_b, 0)),
             bool(np.allclose(f2_b, 0)))
    if flags not in _CACHE:
        _CACHE[flags] = _build(flags)
    nc = _CACHE[flags]

    cos, sinf = _rope_tables()
    kvu_v4 = kvu_w.reshape(R, 2, H, D)
    in_maps = []
    for c in range(8):
        b, half = c // 2, c % 2
        hsl = slice(half * HL, (half + 1) * HL)
        m = {
            "x_loc": np.ascontiguousarray(x[b]),
            "qw_loc": np.ascontiguousarray(
                q_w[:, half * HD:(half + 1) * HD]).astype(BF16),
            "kvd_w": kvd_w.astype(BF16),
            "kvu_k": np.ascontiguousarray(
                kvu_v4[:, 0, hsl, :].reshape(R, HD)).astype(BF16),
            "kvu_v": np.ascontiguousarray(
                kvu_v4[:, 1, hsl, :].reshape(R, HD)).astype(BF16),
            "proj_w_loc": np.ascontiguousarray(
                proj_w[half * HD:(half + 1) * HD, :]).astype(BF16),
            "f1_w_loc": np.ascontiguousarray(
                f1_w[:, half * FH:(half + 1) * FH]).astype(BF16),
            "f2_w_loc": np.ascontiguousarray(
                f2_w[half * FH:(half + 1) * FH, :]).astype(BF16),
            "cos_t": cos.astype(BF16),
            "sinf_t": sinf.astype(BF16),
        }
        if not flags[0]:
            m["ln1_w"], m["ln1_b"] = ln1_w, ln1_b
        if not flags[1]:
            m["kvln_w"], m["kvln_b"] = kvln_w, kvln_b
        if not flags[2]:
            m["ln2_w"], m["ln2_b"] = ln2_w, ln2_b
        if not flags[3]:
            m["proj_b"] = proj_b
        if not flags[4]:
            m["f1_b_loc"] = np.ascontiguousarray(f1_b[half * FH:(half + 1) * FH])
        if not flags[5]:
            m["f2_b"] = f2_b
        in_maps.append(m)

    res = run_bass_kernel_spmd(nc, in_maps, list(range(8)), trace=TRACE)
    kernel.last_result = res
    out = np.stack([res.results[2 * b]["out_loc"] for b in range(B)])
    return out



# revision 58
# speedup vs baseline: 1.0139x; 1.0139x over previous
"""Trainium2 Bass kernel for nn_Block_11897059410591 (MLA transformer block).

Sharding over 8 NeuronCores: core c = (batch b=c//2, head-half h0=(c%2)*8).
Each core computes LN1/kvd/kvu/RoPE for its whole batch, causal attention for
its 8 heads, a partial output projection (contracted over its heads) that is
pair-AllReduced, then the FFN with d_ff split in half across the pair and a
second pair-AllReduce. Both cores of a pair end with the identical full-batch
output; the host keeps the even core's copy.

proj and f2 emit token-major partials so the collective payloads are
[tokens, C] and the residual/LN2/final paths need no PE transposes; the final
output recomputes x + sa + ff from DRAM so nothing is buffered across the
back half. The trailing f2 collective is split 512/512/512/384/128 so the
exposed tail is one 0.25 MB AllReduce.

Scheduling notes (the Tile scheduler orders each engine queue by its own
simulated readiness, so structure must be encoded in emission order + deps):
- a tiny warm-up AllReduce at t=0 absorbs the CC stack's cold start;
- prep is emitted as an explicit two-stage software pipeline (stage A of
  chunk i+1 before stage B of chunk i) so adjacent chunks overlap;
- RoPE runs in all-bf16 on DVE (2x rate) off a flat ACT-evicted psum copy;
- attention emits all qk score groups (64-row PE mode, head pair in
  disjoint row groups) before all av groups (128-row mode) per head-pair;
- back-half tiles whose loads depend on collective outputs take a 1-element
  probe-write reading a gate produced by the last attention eviction, so
  their loads cannot be hoisted into the attention region of the queues
  where a slow AllReduce would head-of-line-block everything behind it.
"""
import sys

if "/opt/trn_rl_repo" not in sys.path:
    sys.path.insert(0, "/opt/trn_rl_repo")

import numpy as np
import ml_dtypes


def _ensure_ntff_hook():
    """antenv.axon_hooks is missing in this image; shim it so
    run_bass_kernel_spmd(trace=True) can capture NTFF profiles."""
    try:
        from antenv import axon_hooks  # noqa: F401
        return
    except ImportError:
        pass
    try:
        import types
        import importlib.util
        m = types.ModuleType("antenv.axon_hooks")
        _hook = [None]
        m.set_axon_ntff_profile_hook = lambda h: _hook.__setitem__(0, h)
        m.get_axon_ntff_profile_hook = lambda: _hook[0]
        sys.modules["antenv.axon_hooks"] = m
        import antenv
        antenv.axon_hooks = m
        spec = importlib.util.spec_from_file_location(
            "_trn_boot_shim", "/root/.axon_site/trn_agent_boot/trn_boot.py")
        tb = importlib.util.module_from_spec(spec)
        spec.loader.exec_module(tb)
        hook = tb._ntff_profile_via_ctypes("/opt/axon/libaxon_pjrt.so")
        m.set_axon_ntff_profile_hook(hook)
    except Exception as e:  # degrade to trace-less operation
        print(f"ntff hook shim failed ({e}); tracing disabled", file=sys.stderr)


_ensure_ntff_hook()

import concourse.bass as bass
import concourse.mybir as mybir
import concourse.tile as tile
from concourse import bacc
from concourse.bass_utils import run_bass_kernel_spmd
from concourse.masks import make_identity

F32 = mybir.dt.float32
BF = mybir.dt.bfloat16
BF16 = ml_dtypes.bfloat16
AF = mybir.ActivationFunctionType
ALU = mybir.AluOpType

B, T, C = 4, 2048, 1024
H, D, R, FF = 16, 64, 512, 4096
HL = 8              # heads per core
HD = HL * D         # 512
FH = FF // 2        # 2048, d_ff half per core
P = 128
NT = T // P         # 16 token chunks
NCC = C // P        # 8 C chunks
NRC = R // P        # 4 R chunks
NSL = 4             # token slabs for the back half
SLB = T // NSL      # 512 tokens per slab
LN_EPS = 1e-5

TRACE = False
_CACHE = {}


def _rope_tables():
    inv_freq = 1.0 / (10000.0 ** (np.arange(0, D, 2, dtype=np.float32) / D))
    t = np.arange(T, dtype=np.float32)
    freqs = np.outer(t, inv_freq)
    emb = np.concatenate([freqs, freqs], axis=-1)  # [T, D]
    cos = np.cos(emb).astype(np.float32)
    sin = np.sin(emb).astype(np.float32)
    sinf = sin.copy()
    sinf[:, : D // 2] = -sinf[:, : D // 2]
    return cos, sinf


def _build(flags):
    (ln1_triv, kvln_triv, ln2_triv, pb0, f1b0, f2b0) = flags
    nc = bacc.Bacc("TRN2", target_bir_lowering=False, debug=False,
                   enable_asserts=False, num_devices=8)

    x_d = nc.dram_tensor("x_loc", [T, C], F32, kind="ExternalInput").ap()
    qw_d = nc.dram_tensor("qw_loc", [C, HD], BF, kind="ExternalInput").ap()
    kvd_d = nc.dram_tensor("kvd_w", [C, R], BF, kind="ExternalInput").ap()
    kvuk_d = nc.dram_tensor("kvu_k", [R, HD], BF, kind="ExternalInput").ap()
    kvuv_d = nc.dram_tensor("kvu_v", [R, HD], BF, kind="ExternalInput").ap()
    pw_d = nc.dram_tensor("proj_w_loc", [HD, C], BF, kind="ExternalInput").ap()
    f1_d = nc.dram_tensor("f1_w_loc", [C, FH], BF, kind="ExternalInput").ap()
    f2_d = nc.dram_tensor("f2_w_loc", [FH, C], BF, kind="ExternalInput").ap()
    cos_d = nc.dram_tensor("cos_t", [T, D], BF, kind="ExternalInput").ap()
    sinf_d = nc.dram_tensor("sinf_t", [T, D], BF, kind="ExternalInput").ap()
    out_d = nc.dram_tensor("out_loc", [T, C], F32, kind="ExternalOutput").ap()

    opt_ins = {}
    if not ln1_triv:
        opt_ins["ln1_w"] = nc.dram_tensor("ln1_w", [C], F32, kind="ExternalInput").ap()
        opt_ins["ln1_b"] = nc.dram_tensor("ln1_b", [C], F32, kind="ExternalInput").ap()
    if not kvln_triv:
        opt_ins["kvln_w"] = nc.dram_tensor("kvln_w", [R], F32, kind="ExternalInput").ap()
        opt_ins["kvln_b"] = nc.dram_tensor("kvln_b", [R], F32, kind="ExternalInput").ap()
    if not ln2_triv:
        opt_ins["ln2_w"] = nc.dram_tensor("ln2_w", [C], F32, kind="ExternalInput").ap()
        opt_ins["ln2_b"] = nc.dram_tensor("ln2_b", [C], F32, kind="ExternalInput").ap()
    if not pb0:
        opt_ins["proj_b"] = nc.dram_tensor("proj_b", [C], F32, kind="ExternalInput").ap()
    if not f1b0:
        opt_ins["f1_b"] = nc.dram_tensor("f1_b_loc", [FH], F32, kind="ExternalInput").ap()
    if not f2b0:
        opt_ins["f2_b"] = nc.dram_tensor("f2_b", [C], F32, kind="ExternalInput").ap()

    # internal DRAM (all collective payloads token-major)
    cc1_in = [nc.dram_tensor(f"cc1_in{n}", [SLB, C], BF).ap()
              for n in range(NSL)]
    cc1_out = [nc.dram_tensor(f"cc1_out{n}", [SLB, C], BF).ap()
               for n in range(NSL)]
    ccw_in = nc.dram_tensor("ccw_in", [P, 2], F32).ap()
    ccw_out = nc.dram_tensor("ccw_out", [P, 2], F32).ap()
    # f2 collective chunks: (token_start, n_tokens); only the last chunk is
    # small (per-op fixed cost ~10us dominates below ~0.5MB, so an evenly
    # fine split would serialize on the CC stream)
    CC2 = [(0, 512), (512, 512), (1024, 512), (1536, 384), (1920, 128)]
    cc2_in = [nc.dram_tensor(f"cc2_in{ci}", [ln, C], BF).ap()
              for ci, (t0, ln) in enumerate(CC2)]
    cc2_out = [nc.dram_tensor(f"cc2_out{ci}", [ln, C], BF).ap()
               for ci, (t0, ln) in enumerate(CC2)]
    groups = [[0, 1], [2, 3], [4, 5], [6, 7]]

    def bcast_free(ap2d, n, width):
        """[P, width] AP -> [P, n, width] with 0-step middle dim."""
        return bass.AP(tensor=ap2d.tensor, offset=ap2d.offset,
                       ap=[ap2d.ap[0], [0, n], [1, width]])

    def ln_stats(pool, src_ap, width, eps_t):
        """Per-partition (mean, rstd) of src_ap [P, width]."""
        ngr = (width + 511) // 512
        st6 = pool.tile([P, ngr, 6], F32, tag="st6")
        sv = src_ap.rearrange("p (g d) -> p g d", g=ngr)
        for g in range(ngr):
            nc.vector.bn_stats(out=st6[:, g, :], in_=sv[:, g, :])
        mv = pool.tile([P, 2], F32, tag="mv")
        nc.vector.bn_aggr(out=mv, in_=st6)
        nc.scalar.activation(out=mv[:, 1:2], in_=mv[:, 1:2], func=AF.Sqrt,
                             bias=eps_t, scale=1.0)
        nc.vector.reciprocal(out=mv[:, 1:2], in_=mv[:, 1:2])
        return mv

    from contextlib import ExitStack
    with tile.TileContext(nc) as tc:
        with ExitStack() as ctx:
            const = ctx.enter_context(tc.tile_pool(name="const", bufs=1))
            ident = const.tile([P, P], BF)
            make_identity(nc, ident)
            eps_t = const.tile([P, 1], F32)
            nc.vector.memset(eps_t, LN_EPS)
            # S^T diagonal causal mask: keep (0) where col(q) >= row(k)
            maskT = const.tile([P, P], F32)
            nc.gpsimd.memset(maskT, 0.0)
            nc.gpsimd.affine_select(out=maskT, in_=maskT, compare_op=ALU.is_ge,
                                    fill=-1e9, base=0, pattern=[[1, P]],
                                    channel_multiplier=-1)
            mask_full = const.tile([P, P], F32)
            nc.vector.memset(mask_full, -1e9)
            # Written at the end of attention; back-half tiles that depend on
            # collective outputs take a 1-element probe-write reading this
            # first, so the scheduler cannot hoist their loads (and the DVE
            # ops behind them) into the attention region of the engine
            # queues, where a slow AllReduce would head-of-line-block it.
            gate_t = const.tile([1, 4], F32, name="gate")
            # [full | diag] combined mask for the kb==q0+1 block pair
            mask2 = const.tile([P, 2, P], F32)
            nc.vector.memset(mask2[:, 0, :], -1e9)
            nc.vector.tensor_copy(out=mask2[:, 1, :], in_=maskT)

            # CC-stream warm-up: a tiny AllReduce issued at t=0 absorbs the
            # collective stack's cold-start cost under the prep phase, so
            # the first real AllReduce runs at steady-state speed.
            ccw_t = const.tile([P, 2], F32)
            nc.vector.memset(ccw_t, 1.0)
            nc.sync.dma_start(out=ccw_in, in_=ccw_t)
            nc.gpsimd.collective_compute(
                "AllReduce", ALU.add, replica_groups=groups,
                ins=[ccw_in], outs=[ccw_out])

            def dram_row_bcast(name, ap1d, width):
                t = const.tile([P, width], F32, name=name)
                src = bass.AP(tensor=ap1d.tensor, offset=ap1d.offset,
                              ap=[[0, P], [1, width]])
                nc.sync.dma_start(out=t, in_=src)
                return t

            ln1_wt = ln1_bt = ln2_wt = ln2_bt = kvln_wt = kvln_bt = None
            if not ln1_triv:
                ln1_wt = dram_row_bcast("ln1w_b", opt_ins["ln1_w"], C)
                ln1_bt = dram_row_bcast("ln1b_b", opt_ins["ln1_b"], C)
            if not kvln_triv:
                kvln_wt = dram_row_bcast("kvlnw_b", opt_ins["kvln_w"], R)
                kvln_bt = dram_row_bcast("kvlnb_b", opt_ins["kvln_b"], R)
            if not ln2_triv:
                ln2_wt = dram_row_bcast("ln2w_b", opt_ins["ln2_w"], C)
                ln2_bt = dram_row_bcast("ln2b_b", opt_ins["ln2_b"], C)
            projb_t = f1b_t = f2b_t = None
            if not pb0:
                # token-major proj output -> bias is a broadcast row [*, C]
                projb_t = dram_row_bcast("projb_b", opt_ins["proj_b"], C)
            if not f1b0:
                f1b_t = const.tile([P, FH // P], F32, name="f1b")
                nc.sync.dma_start(out=f1b_t, in_=opt_ins["f1_b"].rearrange(
                    "(m p) -> p m", p=P))
            if not f2b0:
                f2b_t = dram_row_bcast("f2b_b", opt_ins["f2_b"], C)

            def load_chunks(pool, dram_ap, nk, width, name):
                t = pool.tile([P, nk, width], BF, name=name)
                for k in range(nk):
                    nc.sync.dma_start(out=t[:, k, :],
                                      in_=dram_ap[k * P:(k + 1) * P, :])
                return t

            # ---- long-lived pools (created early; closed last, LIFO) ------
            pool_xs = ctx.enter_context(tc.tile_pool(name="xs2", bufs=3))
            pool_h2T = ctx.enter_context(tc.tile_pool(name="h2T", bufs=2))
            h2T_sl = [None] * NSL
            fivb_pools = {}

            def emit_5b(n):
                """Residual + LN2 + h2T for token slab n. cc1_out is
                token-major so no PE transposes are needed before LN2; only
                h2 -> h2T is transposed (for the f1 matmul rhs), in a second
                pass so the PE doesn't stall on the DVE LN chain."""
                h2T = pool_h2T.tile([P, NCC, SLB], BF, tag="h2T")
                h2T_sl[n] = h2T
                sa_in = fivb_pools["sa_in"].tile([P, SLB // P, C], BF,
                                                 tag="sa_in", name="sa_in")
                # probe-write: orders the loads after attention (see gate_t)
                nc.vector.tensor_copy(out=sa_in[0:1, 0, 0:4], in_=gate_t)
                for itl in range(SLB // P):
                    nc.sync.dma_start(
                        out=sa_in[:, itl, :],
                        in_=cc1_out[n][itl * P:(itl + 1) * P, :])
                h2s = []
                for itl in range(SLB // P):
                    it = n * (SLB // P) + itl
                    sl = slice(it * P, (it + 1) * P)
                    xt = pool_xs.tile([P, C], F32, tag="xt2")
                    nc.sync.dma_start(out=xt, in_=x_d[sl, :])
                    x2t = fivb_pools["x2"].tile([P, C], F32, tag="x2t",
                                                name="x2t")
                    nc.vector.tensor_add(x2t, xt, sa_in[:, itl, :])
                    mv = ln_stats(fivb_pools["st2"], x2t, C, eps_t)
                    h2 = fivb_pools["tok2"].tile([P, C], BF, tag="h2", name="h2")
                    nc.vector.tensor_scalar(out=h2, in0=x2t,
                                            scalar1=mv[:, 0:1],
                                            scalar2=mv[:, 1:2],
                                            op0=ALU.subtract, op1=ALU.mult)
                    if ln2_wt is not None:
                        nc.vector.tensor_mul(h2, h2, ln2_wt)
                        nc.vector.tensor_add(h2, h2, ln2_bt)
                    h2s.append((h2, slice(itl * P, (itl + 1) * P)))
                for h2, lsl in h2s:
                    pt2 = ps_tr1.tile([P, C], BF, tag="p1k")
                    for kc in range(NCC):
                        nc.tensor.transpose(pt2[:, kc * P:(kc + 1) * P],
                                            h2[:, kc * P:(kc + 1) * P],
                                            ident)
                    nc.scalar.copy(
                        out=h2T[:, :, lsl],
                        in_=pt2.rearrange("p (kc t) -> p kc t", kc=NCC))

            # ---------------- qkv/att scope --------------------------------
            mid = ExitStack()
            pool_qkv = mid.enter_context(tc.tile_pool(name="qkv", bufs=1))
            pool_att = mid.enter_context(tc.tile_pool(name="att", bufs=2))
            qT = pool_qkv.tile([P, HL // 2, T], BF)   # [(2h,64d), hp, T]
            kT = pool_qkv.tile([P, HL // 2, T], BF)
            vaug = pool_qkv.tile([P, NT, HL, D + 1], BF)

            # ------- prep (fused per-chunk): LN1 -> q/kv/RoPE --------------
            # One loop over 16 token chunks keeps the PE fed with matmuls
            # from the start (HAM stays warm) and lets DVE LN/RoPE for chunk
            # i+1 overlap PE work for chunk i. hT/ckvT chunks are only used
            # within their iteration, so they rotate in small pools.
            with ExitStack() as prep:
                pool_hT = prep.enter_context(tc.tile_pool(name="hT", bufs=3))
                pool_ckvT = prep.enter_context(tc.tile_pool(name="ckvT", bufs=3))
                pool_xsp = prep.enter_context(tc.tile_pool(name="xs", bufs=4))
                pool_stp = prep.enter_context(tc.tile_pool(name="st", bufs=6))
                pool_tokp = prep.enter_context(tc.tile_pool(name="tok", bufs=4))
                pool_w = prep.enter_context(tc.tile_pool(name="wts", bufs=1))
                pool_cs = prep.enter_context(tc.tile_pool(name="cs", bufs=4))
                pool_ro = prep.enter_context(tc.tile_pool(name="ro", bufs=6))
                # 4 accumulators per chunk: bufs=6 gives 1.5 chunks of
                # cross-chunk pipeline depth (the 3-slot ring serialized
                # chunks almost completely)
                ps_big = prep.enter_context(
                    tc.tile_pool(name="psbig", bufs=6, space="PSUM"))
                ps_tr = prep.enter_context(
                    tc.tile_pool(name="pstr", bufs=2, space="PSUM"))

                # x/cos loads for the first chunks are emitted BEFORE the
                # weight loads so the LN1 chain starts at ~2us instead of
                # queueing behind 4MB of weights on the same DMA queue
                xstash = {}

                def load_x(it):
                    sl = slice(it * P, (it + 1) * P)
                    xt = pool_xsp.tile([P, C], F32)
                    nc.sync.dma_start(out=xt, in_=x_d[sl, :])
                    cos_sb = pool_cs.tile([P, D], BF, tag="cos")
                    nc.sync.dma_start(out=cos_sb, in_=cos_d[sl, :])
                    sinf_sb = pool_cs.tile([P, D], BF, tag="sinf")
                    nc.sync.dma_start(out=sinf_sb, in_=sinf_d[sl, :])
                    xstash[it] = (xt, cos_sb, sinf_sb)

                load_x(0)
                load_x(1)
                kvdw_sb = load_chunks(pool_w, kvd_d, NCC, R, "kvdw")
                qw_sb = load_chunks(pool_w, qw_d, NCC, HD, "qw")
                kvuk_sb = load_chunks(pool_w, kvuk_d, NRC, HD, "kvuk")
                kvuv_sb = load_chunks(pool_w, kvuv_d, NRC, HD, "kvuv")

                def rope(ps, cos_sb, sinf_sb):
                    # ACT evicts psum to a flat bf16 tile (plain write, no
                    # view); all-bf16 DVE ops then run at 2x rate
                    qbf = pool_ro.tile([P, HD], BF, tag="qbf", name="qbf")
                    nc.scalar.copy(out=qbf, in_=ps)
                    psv = qbf.rearrange("p (h d) -> p h d", d=D)
                    t1 = pool_ro.tile([P, HL, D], BF, tag="t1")
                    nc.vector.tensor_mul(t1, psv, bcast_free(cos_sb, HL, D))
                    t2 = pool_ro.tile([P, HL, D], BF, tag="t2")
                    half = D // 2
                    sfv = sinf_sb
                    nc.vector.tensor_mul(
                        t2[:, :, 0:half],
                        bass.AP(tensor=psv.tensor, offset=psv.offset + half,
                                ap=[psv.ap[0], [D, HL], [1, half]]),
                        bass.AP(tensor=sfv.tensor, offset=sfv.offset,
                                ap=[sfv.ap[0], [0, HL], [1, half]]))
                    nc.vector.tensor_mul(
                        t2[:, :, half:D],
                        bass.AP(tensor=psv.tensor, offset=psv.offset,
                                ap=[psv.ap[0], [D, HL], [1, half]]),
                        bass.AP(tensor=sfv.tensor, offset=sfv.offset + half,
                                ap=[sfv.ap[0], [0, HL], [1, half]]))
                    ro = pool_ro.tile([P, HL, D], BF, tag="ro")
                    nc.vector.tensor_add(ro, t1, t2)
                    return ro.rearrange("p h d -> p (h d)")

                def evict_pairs(ro_flat, dstT, sl):
                    # [tok, (2 heads x 64d)] pair-chunks -> dstT[(par,d), hp]
                    for hp in range(HL // 2):
                        pt = ps_tr.tile([P, P], BF, tag="ptr")
                        nc.tensor.transpose(
                            pt, ro_flat[:, hp * P:(hp + 1) * P], ident)
                        if hp % 2 == 0:
                            nc.scalar.copy(out=dstT[:, hp, sl], in_=pt)
                        else:
                            nc.vector.tensor_copy(out=dstT[:, hp, sl], in_=pt)

                # Explicit two-stage software pipeline: stage A of chunk
                # i+1 is emitted before stage B of chunk i, so every engine
                # queue interleaves adjacent chunks (the scheduler's cost
                # model underestimates DVE latency and otherwise serializes
                # the whole per-chunk chain).
                stash = {}

                def stage_a(it):
                    sl = slice(it * P, (it + 1) * P)
                    if it not in xstash:
                        load_x(it)
                    xt, cos_sb, sinf_sb = xstash[it]

                    mv = ln_stats(pool_stp, xt, C, eps_t)
                    ht = pool_tokp.tile([P, C], BF, tag="ht")
                    nc.vector.tensor_scalar(out=ht, in0=xt,
                                            scalar1=mv[:, 0:1], scalar2=mv[:, 1:2],
                                            op0=ALU.subtract, op1=ALU.mult)
                    if ln1_wt is not None:
                        nc.vector.tensor_mul(ht, ht, ln1_wt)
                        nc.vector.tensor_add(ht, ht, ln1_bt)
                    hTc = pool_hT.tile([P, NCC, P], BF, tag="hTc")
                    for kc in range(NCC):
                        pt = ps_tr.tile([P, P], BF, tag="ptr")
                        nc.tensor.transpose(pt, ht[:, kc * P:(kc + 1) * P], ident)
                        if kc % 2 == 0:
                            nc.scalar.copy(out=hTc[:, kc, :], in_=pt)
                        else:
                            nc.vector.tensor_copy(out=hTc[:, kc, :], in_=pt)

                    psq = ps_big.tile([P, HD], F32, tag="psb")
                    for kc in range(NCC):
                        nc.tensor.matmul(psq, lhsT=hTc[:, kc, :],
                                         rhs=qw_sb[:, kc, :],
                                         start=(kc == 0), stop=(kc == NCC - 1))
                    ps = ps_big.tile([P, R], F32, tag="psb")
                    for kc in range(NCC):
                        nc.tensor.matmul(ps, lhsT=hTc[:, kc, :],
                                         rhs=kvdw_sb[:, kc, :],
                                         start=(kc == 0), stop=(kc == NCC - 1))
                    stash[it] = (psq, ps, cos_sb, sinf_sb)

                def stage_b(it):
                    sl = slice(it * P, (it + 1) * P)
                    psq, ps, cos_sb, sinf_sb = stash.pop(it)
                    ro_q = rope(psq, cos_sb, sinf_sb)

                    mv = ln_stats(pool_stp, ps, R, eps_t)
                    ct = pool_tokp.tile([P, R], BF, tag="ckvtok")
                    nc.vector.tensor_scalar(out=ct, in0=ps,
                                            scalar1=mv[:, 0:1], scalar2=mv[:, 1:2],
                                            op0=ALU.subtract, op1=ALU.mult)
                    if kvln_wt is not None:
                        nc.vector.tensor_mul(ct, ct, kvln_wt)
                        nc.vector.tensor_add(ct, ct, kvln_bt)

                    evict_pairs(ro_q, qT, sl)

                    ckvTc = pool_ckvT.tile([P, NRC, P], BF, tag="ckvTc")
                    for rc in range(NRC):
                        pt = ps_tr.tile([P, P], BF, tag="ptr")
                        nc.tensor.transpose(pt, ct[:, rc * P:(rc + 1) * P], ident)
                        if rc % 2 == 0:
                            nc.scalar.copy(out=ckvTc[:, rc, :], in_=pt)
                        else:
                            nc.vector.tensor_copy(out=ckvTc[:, rc, :], in_=pt)

                    psk = ps_big.tile([P, HD], F32, tag="psb")
                    for rc in range(NRC):
                        nc.tensor.matmul(psk, lhsT=ckvTc[:, rc, :],
                                         rhs=kvuk_sb[:, rc, :],
                                         start=(rc == 0), stop=(rc == NRC - 1))
                    ro_k = rope(psk, cos_sb, sinf_sb)

                    psv = ps_big.tile([P, HD], F32, tag="psb")
                    for rc in range(NRC):
                        nc.tensor.matmul(psv, lhsT=ckvTc[:, rc, :],
                                         rhs=kvuv_sb[:, rc, :],
                                         start=(rc == 0), stop=(rc == NRC - 1))

                    evict_pairs(ro_k, kT, sl)

                    nc.vector.memset(vaug[:, it, :, D:D + 1], 1.0)
                    nc.scalar.copy(out=vaug[:, it, :, 0:D],
                                   in_=psv.rearrange("p (h d) -> p h d", d=D))

                stage_a(0)
                stage_a(1)
                for it in range(NT):
                    if it + 2 < NT:
                        stage_a(it + 2)
                    stage_b(it)

            # ---- Phase 3+5a+5b: attention / proj / residual interleaved ---
            with ExitStack() as attn:
                pool_p = attn.enter_context(tc.tile_pool(name="pexp", bufs=16))
                pool_s = attn.enter_context(tc.tile_pool(name="srow", bufs=4))
                pool_bc = attn.enter_context(tc.tile_pool(name="bc", bufs=4))
                pool_pw = attn.enter_context(tc.tile_pool(name="pw", bufs=1))
                pool_sa = attn.enter_context(tc.tile_pool(name="sa", bufs=3))
                ps_s = attn.enter_context(
                    tc.tile_pool(name="pss", bufs=4, space="PSUM"))
                ps_o = attn.enter_context(
                    tc.tile_pool(name="pso", bufs=1, space="PSUM"))
                ps_pj = attn.enter_context(
                    tc.tile_pool(name="pspj", bufs=1, space="PSUM"))
                pw_sb = load_chunks(pool_pw, pw_d, NRC, C, "pw")
                NPAIR = NT // 2
                att_cur = [None]  # per-slab [(2h,64d), hp, SLB] tile

                def proj_slab(n):
                    # token-major partial: out[tok, C] = att[:, hp, tok].T @ pw
                    att = att_cur[0]
                    for itl in range(SLB // P):
                        tsl = slice(itl * P, (itl + 1) * P)
                        ps = ps_pj.tile([P, C], F32, tag="pspj")
                        for half in range(2):
                            hsl = slice(half * 512, (half + 1) * 512)
                            for hp in range(NRC):
                                nc.tensor.matmul(ps[:, hsl],
                                                 lhsT=att[:, hp, tsl],
                                                 rhs=pw_sb[:, hp, hsl],
                                                 start=(hp == 0),
                                                 stop=(hp == NRC - 1))
                        sa_t = pool_sa.tile([P, C], BF, tag="sat")
                        if projb_t is not None:
                            nc.vector.tensor_add(sa_t, ps, projb_t)
                        else:
                            nc.scalar.copy(out=sa_t, in_=ps)
                        nc.sync.dma_start(out=cc1_in[n][tsl, :], in_=sa_t)
                    nc.gpsimd.collective_compute(
                        "AllReduce", ALU.add, replica_groups=groups,
                        ins=[cc1_in[n]], outs=[cc1_out[n]])

                # Per head-pair, interleave the two heads (PE row groups 0-63
                # and 64-127) and pipeline qk of group g+1 ahead of av of
                # group g so the Exp never stalls the PE.
                GRP = 2  # key-blocks per score group
                for pi in range(NPAIR):
                    if pi % 2 == 0:
                        att_cur[0] = pool_att.tile([P, NRC, SLB], BF,
                                                   tag="attsl", name="attsl")
                    q0 = 2 * pi            # first q-block of pair
                    qsl = slice(q0 * P, (q0 + 2) * P)     # 256 queries
                    asl = slice((pi % 2) * 2 * P, (pi % 2 + 1) * 2 * P)
                    nkb = 2 * pi + 2
                    ngr = nkb // GRP

                    for hp in range(HL // 2):
                        poA = ps_o.tile([P, 2 * P], F32, tag="poA", name="poA")
                        poB = ps_o.tile([P, 2 * P], F32, tag="poB", name="poB")
                        pos = {0: poA, 64: poB}

                        def qk_group(g, hr):
                            pss = ps_s.tile([P, GRP * 2 * P], F32, tag="pss")
                            for j in range(GRP):
                                kb = g * GRP + j
                                jsl = slice(j * 2 * P, (j + 1) * 2 * P)
                                nc.tensor.matmul(
                                    pss[:, jsl],
                                    lhsT=kT[hr:hr + 64, hp, kb * P:(kb + 1) * P],
                                    rhs=qT[hr:hr + 64, hp, qsl],
                                    start=True, stop=True)
                                if kb == q0:
                                    nc.vector.tensor_add(
                                        pss[:, j * 2 * P:j * 2 * P + P],
                                        pss[:, j * 2 * P:j * 2 * P + P], maskT)
                                elif kb == q0 + 1:
                                    nc.vector.tensor_add(
                                        pss[:, jsl], pss[:, jsl],
                                        mask2.rearrange("p a b -> p (a b)"))
                            pexp = pool_p.tile([P, GRP * 2 * P], BF, tag="pexp")
                            nc.scalar.activation(out=pexp, in_=pss,
                                                 func=AF.Exp,
                                                 scale=float(D) ** -0.5)
                            return pexp

                        def av_group(g, hr, pexp):
                            po = pos[hr]
                            h = 2 * hp + (1 if hr else 0)
                            for j in range(GRP):
                                kb = g * GRP + j
                                nc.tensor.matmul(
                                    po[0:D + 1, :],
                                    lhsT=vaug[:, kb, h, :],
                                    rhs=pexp[:, j * 2 * P:(j + 1) * 2 * P],
                                    start=(kb == 0), stop=(kb == nkb - 1))

                        # all qk groups first (64-row array mode, the two
                        # heads in disjoint row groups), then all av groups
                        # (128-row mode): two mode switches per head-pair
                        # instead of two per group
                        pexps = []
                        for g in range(ngr):
                            pexps.append((qk_group(g, 0), qk_group(g, 64)))
                        for g in range(ngr):
                            av_group(g, 0, pexps[g][0])
                            av_group(g, 64, pexps[g][1])

                        # normalization fused into eviction
                        for hr in (0, 64):
                            po = pos[hr]
                            srow = pool_s.tile([1, 2 * P], F32, tag="srow")
                            nc.vector.tensor_copy(out=srow, in_=po[D:D + 1, :])
                            bc = pool_bc.tile([64, 2 * P], F32, tag="bc")
                            nc.gpsimd.partition_broadcast(out_ap=bc, in_ap=srow)
                            rc_ = pool_bc.tile([64, 2 * P], F32, tag="rc")
                            nc.vector.reciprocal_approx_fast(out=rc_, in_=bc)
                            nc.vector.tensor_mul(
                                att_cur[0][hr:hr + 64, hp, asl],
                                po[0:D, :], rc_)
                    if pi % 2 == 1:
                        proj_slab(pi // 2)

                # gate: depends on the last pair's attention eviction
                nc.vector.tensor_copy(out=gate_t,
                                      in_=att_cur[0][0:1, NRC - 1,
                                                     SLB - 4:SLB])

            mid.close()  # release qT/kT/vaug/att SBUF

            # ------- Back half: FFN + second AllReduce + final -------------
            with ExitStack() as bh:
                pool_fw = bh.enter_context(tc.tile_pool(name="fw", bufs=1))
                pool_g = bh.enter_context(tc.tile_pool(name="gT", bufs=2))
                pool_ev = bh.enter_context(tc.tile_pool(name="ev", bufs=3))
                pool_ffin = bh.enter_context(tc.tile_pool(name="ffin", bufs=3))
                pool_of = bh.enter_context(tc.tile_pool(name="of", bufs=3))
                ps_f = bh.enter_context(
                    tc.tile_pool(name="psf", bufs=2, space="PSUM"))
                ps_f2 = bh.enter_context(
                    tc.tile_pool(name="psf2", bufs=2, space="PSUM"))
                ps_tr1 = bh.enter_context(
                    tc.tile_pool(name="pstr1", bufs=2, space="PSUM"))
                fivb_pools["sa_in"] = bh.enter_context(
                    tc.tile_pool(name="sain", bufs=2))
                fivb_pools["x2"] = bh.enter_context(
                    tc.tile_pool(name="x2", bufs=2))
                fivb_pools["st2"] = bh.enter_context(
                    tc.tile_pool(name="st2", bufs=4))
                fivb_pools["tok2"] = bh.enter_context(
                    tc.tile_pool(name="tok2", bufs=4))

                # f1/f2 weight loads first: the DMAs overlap the attention
                # tail (fresh pool space, no released-zone wait on hot tiles)
                f1w_sb = load_chunks(pool_fw, f1_d, NCC, FH, "f1w")
                f2w_sb = load_chunks(pool_fw, f2_d, FH // P, C, "f2w")

                gT_sl = [None] * NSL
                NIT = SLB // P  # 4 token blocks per slab

                def f1_slab(n):
                    h2T = h2T_sl[n]
                    gT = pool_g.tile([P, FH // P, SLB], BF, tag="gT")
                    gT_sl[n] = gT
                    for m in range(FH // P):
                        ps = ps_f.tile([P, SLB], F32, tag="psf1")
                        for kc in range(NCC):
                            nc.tensor.matmul(ps,
                                             lhsT=f1w_sb[:, kc, m * P:(m + 1) * P],
                                             rhs=h2T[:, kc, :],
                                             start=(kc == 0),
                                             stop=(kc == NCC - 1))
                        if f1b_t is not None:
                            nc.scalar.activation(out=gT[:, m, :], in_=ps,
                                                 func=AF.Gelu,
                                                 bias=f1b_t[:, m:m + 1],
                                                 scale=1.0)
                        else:
                            nc.scalar.activation(out=gT[:, m, :], in_=ps,
                                                 func=AF.Gelu, scale=1.0)

                def cc2_home(it):
                    t = it * P
                    for ci, (t0, ln) in enumerate(CC2):
                        if t0 <= t < t0 + ln:
                            return ci, t - t0
                    raise AssertionError(it)

                def f2_block(it):
                    """f2 partial for global 128-token block it, token-major."""
                    n, itl = it // NIT, it % NIT
                    gT = gT_sl[n]
                    tsl = slice(itl * P, (itl + 1) * P)
                    ps = ps_f2.tile([P, C], F32, tag="psf2")
                    for half in range(2):
                        hsl = slice(half * 512, (half + 1) * 512)
                        for kf in range(FH // P):
                            nc.tensor.matmul(ps[:, hsl],
                                             lhsT=gT[:, kf, tsl],
                                             rhs=f2w_sb[:, kf, hsl],
                                             start=(kf == 0),
                                             stop=(kf == FH // P - 1))
                    ev = pool_ev.tile([P, C], BF, tag="ffev")
                    if f2b_t is not None:
                        nc.vector.tensor_add(ev, ps, f2b_t)
                    else:
                        nc.scalar.copy(out=ev, in_=ps)
                    ci, row = cc2_home(it)
                    nc.sync.dma_start(out=cc2_in[ci][row:row + P, :], in_=ev)

                def cc2_launch(ci):
                    nc.gpsimd.collective_compute(
                        "AllReduce", ALU.add, replica_groups=groups,
                        ins=[cc2_in[ci]], outs=[cc2_out[ci]])

                def emit_final(ci):
                    """out = x + sa + ff for collective chunk ci (no PE)."""
                    t0, ln = CC2[ci]
                    for j in range(ln // P):
                        itg = t0 // P + j
                        sl = slice(itg * P, (itg + 1) * P)
                        xt = pool_xs.tile([P, C], F32, tag="xt2")
                        nc.sync.dma_start(out=xt, in_=x_d[sl, :])
                        saf = pool_ffin.tile([P, C], BF, tag="sfin")
                        nc.vector.tensor_copy(out=saf[0:1, 0:4], in_=gate_t)
                        nc.sync.dma_start(
                            out=saf,
                            in_=cc1_out[itg // NIT][(itg % NIT) * P:
                                                    (itg % NIT + 1) * P, :])
                        ff = pool_ffin.tile([P, C], BF, tag="ffin")
                        nc.vector.tensor_copy(out=ff[0:1, 0:4], in_=gate_t)
                        nc.sync.dma_start(out=ff,
                                          in_=cc2_out[ci][j * P:(j + 1) * P, :])
                        ot = pool_of.tile([P, C], F32, tag="of")
                        nc.vector.tensor_add(ot, xt, saf)
                        nc.gpsimd.tensor_add(ot, ot, ff)
                        nc.sync.dma_start(out=out_d[sl, :], in_=ot)

                emit_5b(0)
                f1_slab(0)
                emit_5b(1)
                for it in range(0, 4):
                    f2_block(it)
                cc2_launch(0)
                f1_slab(1)
                emit_5b(2)
                for it in range(4, 8):
                    f2_block(it)
                cc2_launch(1)
                emit_final(0)
                f1_slab(2)
                emit_5b(3)
                for it in range(8, 12):
                    f2_block(it)
                cc2_launch(2)
                emit_final(1)
                f1_slab(3)
                f2_block(12)
                f2_block(13)
                f2_block(14)
                cc2_launch(3)
                f2_block(15)
                cc2_launch(4)
                emit_final(2)
                emit_final(3)
                emit_final(4)

    nc.compile()
    return nc


def kernel(**inputs):
    x = np.asarray(inputs["x"], dtype=np.float32)
    q_w = np.asarray(inputs["q_w"], dtype=np.float32)
    kvd_w = np.asarray(inputs["kvd_w"], dtype=np.float32)
    kvu_w = np.asarray(inputs["kvu_w"], dtype=np.float32)
    proj_w = np.asarray(inputs["proj_w"], dtype=np.float32)
    f1_w = np.asarray(inputs["f1_w"], dtype=np.float32)
    f2_w = np.asarray(inputs["f2_w"], dtype=np.float32)
    ln1_w = np.asarray(inputs["ln1_w"], dtype=np.float32)
    ln1_b = np.asarray(inputs["ln1_b"], dtype=np.float32)
    ln2_w = np.asarray(inputs["ln2_w"], dtype=np.float32)
    ln2_b = np.asarray(inputs["ln2_b"], dtype=np.float32)
    kvln_w = np.asarray(inputs["kvln_w"], dtype=np.float32)
    kvln_b = np.asarray(inputs["kvln_b"], dtype=np.float32)
    proj_b = np.asarray(inputs["proj_b"], dtype=np.float32)
    f1_b = np.asarray(inputs["f1_b"], dtype=np.float32)
    f2_b = np.asarray(inputs["f2_b"], dtype=np.float32)

    flags = (bool(np.allclose(ln1_w, 1) and np.allclose(ln1_b, 0)),
             bool(np.allclose(kvln_w, 1) and np.allclose(kvln_b, 0)),
             bool(np.allclose(ln2_w, 1) and np.allclose(ln2_b, 0)),
             bool(np.allclose(proj_b, 0)),
             bool(np.allclose(f1_b, 0)),
             bool(np.allclose(f2_b, 0)))
    if flags not in _CACHE:
        _CACHE[flags] = _build(flags)
    nc = _CACHE[flags]

    cos, sinf = _rope_tables()
    kvu_v4 = kvu_w.reshape(R, 2, H, D)
    in_maps = []
    for c in range(8):
        b, half = c // 2, c % 2
        hsl = slice(half * HL, (half + 1) * HL)
        m = {
            "x_loc": np.ascontiguousarray(x[b]),
            "qw_loc": np.ascontiguousarray(
                q_w[:, half * HD:(half + 1) * HD]).astype(BF16),
            "kvd_w": kvd_w.astype(BF16),
            "kvu_k": np.ascontiguousarray(
                kvu_v4[:, 0, hsl, :].reshape(R, HD)).astype(BF16),
            "kvu_v": np.ascontiguousarray(
                kvu_v4[:, 1, hsl, :].reshape(R, HD)).astype(BF16),
            "proj_w_loc": np.ascontiguousarray(
                proj_w[half * HD:(half + 1) * HD, :]).astype(BF16),
            "f1_w_loc": np.ascontiguousarray(
                f1_w[:, half * FH:(half + 1) * FH]).astype(BF16),
            "f2_w_loc": np.ascontiguousarray(
                f2_w[half * FH:(half + 1) * FH, :]).astype(BF16),
            "cos_t": cos.astype(BF16),
            "sinf_t": sinf.astype(BF16),
        }
        if not flags[0]:
            m["ln1_w"], m["ln1_b"] = ln1_w, ln1_b
        if not flags[1]:
            m["kvln_w"], m["kvln_b"] = kvln_w, kvln_b
        if not flags[2]:
            m["ln2_w"], m["ln2_b"] = ln2_w, ln2_b
        if not flags[3]:
            m["proj_b"] = proj_b
        if not flags[4]:
            m["f1_b_loc"] = np.ascontiguousarray(f1_b[half * FH:(half + 1) * FH])
        if not flags[5]:
            m["f2_b"] = f2_b
        in_maps.append(m)

    res = run_bass_kernel_spmd(nc, in_maps, list(range(8)), trace=TRACE)
    kernel.last_result = res
    out = np.stack([res.results[2 * b]["out_loc"] for b in range(B)])
    return out



# revision 59
# speedup vs baseline: 1.1707x; 1.1546x over previous
"""Trainium2 Bass kernel for nn_Block_11897059410591 (MLA transformer block).

Sharding over 8 NeuronCores: core c = (batch b=c//2, head-half h0=(c%2)*8).
Each core computes LN1/kvd/kvu/RoPE for its whole batch, causal attention for
its 8 heads, a partial output projection (contracted over its heads) that is
pair-AllReduced, then the FFN with d_ff split in half across the pair and a
second pair-AllReduce. Both cores of a pair end with the identical full-batch
output; the host keeps the even core's copy.

proj and f2 emit token-major partials so the collective payloads are
[tokens, C] and the residual/LN2/final paths need no PE transposes; the final
output recomputes x + sa + ff from DRAM so nothing is buffered across the
back half. The trailing f2 collective is split 512/512/512/384/128 so the
exposed tail is one 0.25 MB AllReduce.

Scheduling notes (the Tile scheduler orders each engine queue by its own
simulated readiness, so structure must be encoded in emission order + deps):
- a tiny warm-up AllReduce at t=0 absorbs the CC stack's cold start;
- prep is emitted as an explicit two-stage software pipeline (stage A of
  chunk i+1 before stage B of chunk i) so adjacent chunks overlap;
- RoPE runs in all-bf16 on DVE (2x rate) off a flat ACT-evicted psum copy;
- attention emits all qk score groups (64-row PE mode, head pair in
  disjoint row groups) before all av groups (128-row mode) per head-pair;
- back-half tiles whose loads depend on collective outputs take a 1-element
  probe-write reading a gate produced by the last attention eviction, so
  their loads cannot be hoisted into the attention region of the queues
  where a slow AllReduce would head-of-line-block everything behind it.
"""
import sys

if "/opt/trn_rl_repo" not in sys.path:
    sys.path.insert(0, "/opt/trn_rl_repo")

import numpy as np
import ml_dtypes


def _ensure_ntff_hook():
    """antenv.axon_hooks is missing in this image; shim it so
    run_bass_kernel_spmd(trace=True) can capture NTFF profiles."""
    try:
        from antenv import axon_hooks  # noqa: F401
        return
    except ImportError:
        pass
    try:
        import types
        import importlib.util
        m = types.ModuleType("antenv.axon_hooks")
        _hook = [None]
        m.set_axon_ntff_profile_hook = lambda h: _hook.__setitem__(0, h)
        m.get_axon_ntff_profile_hook = lambda: _hook[0]
        sys.modules["antenv.axon_hooks"] = m
        import antenv
        antenv.axon_hooks = m
        spec = importlib.util.spec_from_file_location(
            "_trn_boot_shim", "/root/.axon_site/trn_agent_boot/trn_boot.py")
        tb = importlib.util.module_from_spec(spec)
        spec.loader.exec_module(tb)
        hook = tb._ntff_profile_via_ctypes("/opt/axon/libaxon_pjrt.so")
        m.set_axon_ntff_profile_hook(hook)
    except Exception as e:  # degrade to trace-less operation
        print(f"ntff hook shim failed ({e}); tracing disabled", file=sys.stderr)


_ensure_ntff_hook()

import concourse.bass as bass
import concourse.mybir as mybir
import concourse.tile as tile
from concourse import bacc
from concourse.bass_utils import run_bass_kernel_spmd
from concourse.masks import make_identity

F32 = mybir.dt.float32
BF = mybir.dt.bfloat16
BF16 = ml_dtypes.bfloat16
AF = mybir.ActivationFunctionType
ALU = mybir.AluOpType

B, T, C = 4, 2048, 1024
H, D, R, FF = 16, 64, 512, 4096
HL = 8              # heads per core
HD = HL * D         # 512
FH = FF // 2        # 2048, d_ff half per core
P = 128
NT = T // P         # 16 token chunks
NCC = C // P        # 8 C chunks
NRC = R // P        # 4 R chunks
NSL = 4             # token slabs for the back half
SLB = T // NSL      # 512 tokens per slab
LN_EPS = 1e-5

TRACE = False
_CACHE = {}


def _rope_tables():
    inv_freq = 1.0 / (10000.0 ** (np.arange(0, D, 2, dtype=np.float32) / D))
    t = np.arange(T, dtype=np.float32)
    freqs = np.outer(t, inv_freq)
    emb = np.concatenate([freqs, freqs], axis=-1)  # [T, D]
    cos = np.cos(emb).astype(np.float32)
    sin = np.sin(emb).astype(np.float32)
    sinf = sin.copy()
    sinf[:, : D // 2] = -sinf[:, : D // 2]
    return cos, sinf


def _build(flags):
    (ln1_triv, kvln_triv, ln2_triv, pb0, f1b0, f2b0) = flags
    nc = bacc.Bacc("TRN2", target_bir_lowering=False, debug=False,
                   enable_asserts=False, num_devices=8)

    x_d = nc.dram_tensor("x_loc", [T, C], F32, kind="ExternalInput").ap()
    qw_d = nc.dram_tensor("qw_loc", [C, HD], BF, kind="ExternalInput").ap()
    kvd_d = nc.dram_tensor("kvd_w", [C, R], BF, kind="ExternalInput").ap()
    kvuk_d = nc.dram_tensor("kvu_k", [R, HD], BF, kind="ExternalInput").ap()
    kvuv_d = nc.dram_tensor("kvu_v", [R, HD], BF, kind="ExternalInput").ap()
    pw_d = nc.dram_tensor("proj_w_loc", [HD, C], BF, kind="ExternalInput").ap()
    f1_d = nc.dram_tensor("f1_w_loc", [C, FH], BF, kind="ExternalInput").ap()
    f2_d = nc.dram_tensor("f2_w_loc", [FH, C], BF, kind="ExternalInput").ap()
    cos_d = nc.dram_tensor("cos_t", [T, D], BF, kind="ExternalInput").ap()
    sinf_d = nc.dram_tensor("sinf_t", [T, D], BF, kind="ExternalInput").ap()
    out_d = nc.dram_tensor("out_loc", [T, C], F32, kind="ExternalOutput").ap()

    opt_ins = {}
    if not ln1_triv:
        opt_ins["ln1_w"] = nc.dram_tensor("ln1_w", [C], F32, kind="ExternalInput").ap()
        opt_ins["ln1_b"] = nc.dram_tensor("ln1_b", [C], F32, kind="ExternalInput").ap()
    if not kvln_triv:
        opt_ins["kvln_w"] = nc.dram_tensor("kvln_w", [R], F32, kind="ExternalInput").ap()
        opt_ins["kvln_b"] = nc.dram_tensor("kvln_b", [R], F32, kind="ExternalInput").ap()
    if not ln2_triv:
        opt_ins["ln2_w"] = nc.dram_tensor("ln2_w", [C], F32, kind="ExternalInput").ap()
        opt_ins["ln2_b"] = nc.dram_tensor("ln2_b", [C], F32, kind="ExternalInput").ap()
    if not pb0:
        opt_ins["proj_b"] = nc.dram_tensor("proj_b", [C], F32, kind="ExternalInput").ap()
    if not f1b0:
        opt_ins["f1_b"] = nc.dram_tensor("f1_b_loc", [FH], F32, kind="ExternalInput").ap()
    if not f2b0:
        opt_ins["f2_b"] = nc.dram_tensor("f2_b", [C], F32, kind="ExternalInput").ap()

    # internal DRAM (all collective payloads token-major)
    cc1_in = [nc.dram_tensor(f"cc1_in{n}", [SLB, C], BF).ap()
              for n in range(NSL)]
    cc1_out = [nc.dram_tensor(f"cc1_out{n}", [SLB, C], BF).ap()
               for n in range(NSL)]
    ccw_in = nc.dram_tensor("ccw_in", [P, 2], F32).ap()
    ccw_out = nc.dram_tensor("ccw_out", [P, 2], F32).ap()
    # f2 collective chunks: (token_start, n_tokens); only the last chunk is
    # small (per-op fixed cost ~10us dominates below ~0.5MB, so an evenly
    # fine split would serialize on the CC stream)
    CC2 = [(0, 512), (512, 512), (1024, 512), (1536, 384), (1920, 128)]
    cc2_in = [nc.dram_tensor(f"cc2_in{ci}", [ln, C], BF).ap()
              for ci, (t0, ln) in enumerate(CC2)]
    cc2_out = [nc.dram_tensor(f"cc2_out{ci}", [ln, C], BF).ap()
               for ci, (t0, ln) in enumerate(CC2)]
    groups = [[0, 1], [2, 3], [4, 5], [6, 7]]

    def bcast_free(ap2d, n, width):
        """[P, width] AP -> [P, n, width] with 0-step middle dim."""
        return bass.AP(tensor=ap2d.tensor, offset=ap2d.offset,
                       ap=[ap2d.ap[0], [0, n], [1, width]])

    def ln_stats(pool, src_ap, width, eps_t):
        """Per-partition (mean, rstd) of src_ap [P, width]."""
        ngr = (width + 511) // 512
        st6 = pool.tile([P, ngr, 6], F32, tag="st6")
        sv = src_ap.rearrange("p (g d) -> p g d", g=ngr)
        for g in range(ngr):
            nc.vector.bn_stats(out=st6[:, g, :], in_=sv[:, g, :])
        mv = pool.tile([P, 2], F32, tag="mv")
        nc.vector.bn_aggr(out=mv, in_=st6)
        nc.scalar.activation(out=mv[:, 1:2], in_=mv[:, 1:2], func=AF.Sqrt,
                             bias=eps_t, scale=1.0)
        nc.vector.reciprocal(out=mv[:, 1:2], in_=mv[:, 1:2])
        return mv

    from contextlib import ExitStack
    with tile.TileContext(nc) as tc:
        with ExitStack() as ctx:
            const = ctx.enter_context(tc.tile_pool(name="const", bufs=1))
            ident = const.tile([P, P], BF)
            make_identity(nc, ident)
            eps_t = const.tile([P, 1], F32)
            nc.vector.memset(eps_t, LN_EPS)
            # S^T diagonal causal mask: keep (0) where col(q) >= row(k)
            maskT = const.tile([P, P], F32)
            nc.gpsimd.memset(maskT, 0.0)
            nc.gpsimd.affine_select(out=maskT, in_=maskT, compare_op=ALU.is_ge,
                                    fill=-1e9, base=0, pattern=[[1, P]],
                                    channel_multiplier=-1)
            mask_full = const.tile([P, P], F32)
            nc.vector.memset(mask_full, -1e9)
            # Written at the end of attention; back-half tiles that depend on
            # collective outputs take a 1-element probe-write reading this
            # first, so the scheduler cannot hoist their loads (and the DVE
            # ops behind them) into the attention region of the engine
            # queues, where a slow AllReduce would head-of-line-block it.
            gate_t = const.tile([1, 4], F32, name="gate")
            # [full | diag] combined mask for the kb==q0+1 block pair
            mask2 = const.tile([P, 2, P], F32)
            nc.vector.memset(mask2[:, 0, :], -1e9)
            nc.vector.tensor_copy(out=mask2[:, 1, :], in_=maskT)

            # CC-stream warm-up: a tiny AllReduce issued at t=0 absorbs the
            # collective stack's cold-start cost under the prep phase, so
            # the first real AllReduce runs at steady-state speed.
            ccw_t = const.tile([P, 2], F32)
            nc.vector.memset(ccw_t, 1.0)
            nc.sync.dma_start(out=ccw_in, in_=ccw_t)
            nc.gpsimd.collective_compute(
                "AllReduce", ALU.add, replica_groups=groups,
                ins=[ccw_in], outs=[ccw_out])

            def dram_row_bcast(name, ap1d, width):
                t = const.tile([P, width], F32, name=name)
                src = bass.AP(tensor=ap1d.tensor, offset=ap1d.offset,
                              ap=[[0, P], [1, width]])
                nc.sync.dma_start(out=t, in_=src)
                return t

            ln1_wt = ln1_bt = ln2_wt = ln2_bt = kvln_wt = kvln_bt = None
            if not ln1_triv:
                ln1_wt = dram_row_bcast("ln1w_b", opt_ins["ln1_w"], C)
                ln1_bt = dram_row_bcast("ln1b_b", opt_ins["ln1_b"], C)
            if not kvln_triv:
                kvln_wt = dram_row_bcast("kvlnw_b", opt_ins["kvln_w"], R)
                kvln_bt = dram_row_bcast("kvlnb_b", opt_ins["kvln_b"], R)
            if not ln2_triv:
                ln2_wt = dram_row_bcast("ln2w_b", opt_ins["ln2_w"], C)
                ln2_bt = dram_row_bcast("ln2b_b", opt_ins["ln2_b"], C)
            projb_t = f1b_t = f2b_t = None
            if not pb0:
                # token-major proj output -> bias is a broadcast row [*, C]
                projb_t = dram_row_bcast("projb_b", opt_ins["proj_b"], C)
            if not f1b0:
                f1b_t = const.tile([P, FH // P], F32, name="f1b")
                nc.sync.dma_start(out=f1b_t, in_=opt_ins["f1_b"].rearrange(
                    "(m p) -> p m", p=P))
            if not f2b0:
                f2b_t = dram_row_bcast("f2b_b", opt_ins["f2_b"], C)

            def load_chunks(pool, dram_ap, nk, width, name):
                t = pool.tile([P, nk, width], BF, name=name)
                for k in range(nk):
                    nc.sync.dma_start(out=t[:, k, :],
                                      in_=dram_ap[k * P:(k + 1) * P, :])
                return t

            # ---- long-lived pools (created early; closed last, LIFO) ------
            pool_xs = ctx.enter_context(tc.tile_pool(name="xs2", bufs=3))
            pool_h2T = ctx.enter_context(tc.tile_pool(name="h2T", bufs=2))
            h2T_sl = [None] * NSL
            fivb_pools = {}

            def emit_5b(n):
                """Residual + LN2 + h2T for token slab n. cc1_out is
                token-major so no PE transposes are needed before LN2; only
                h2 -> h2T is transposed (for the f1 matmul rhs), in a second
                pass so the PE doesn't stall on the DVE LN chain."""
                h2T = pool_h2T.tile([P, NCC, SLB], BF, tag="h2T")
                h2T_sl[n] = h2T
                sa_in = fivb_pools["sa_in"].tile([P, SLB // P, C], BF,
                                                 tag="sa_in", name="sa_in")
                # probe-write: orders the loads after attention (see gate_t)
                nc.vector.tensor_copy(out=sa_in[0:1, 0, 0:4], in_=gate_t)
                for itl in range(SLB // P):
                    nc.sync.dma_start(
                        out=sa_in[:, itl, :],
                        in_=cc1_out[n][itl * P:(itl + 1) * P, :])
                h2s = []
                for itl in range(SLB // P):
                    it = n * (SLB // P) + itl
                    sl = slice(it * P, (it + 1) * P)
                    xt = pool_xs.tile([P, C], F32, tag="xt2")
                    nc.sync.dma_start(out=xt, in_=x_d[sl, :])
                    x2t = fivb_pools["x2"].tile([P, C], F32, tag="x2t",
                                                name="x2t")
                    nc.vector.tensor_add(x2t, xt, sa_in[:, itl, :])
                    mv = ln_stats(fivb_pools["st2"], x2t, C, eps_t)
                    h2 = fivb_pools["tok2"].tile([P, C], BF, tag="h2", name="h2")
                    nc.vector.tensor_scalar(out=h2, in0=x2t,
                                            scalar1=mv[:, 0:1],
                                            scalar2=mv[:, 1:2],
                                            op0=ALU.subtract, op1=ALU.mult)
                    if ln2_wt is not None:
                        nc.vector.tensor_mul(h2, h2, ln2_wt)
                        nc.vector.tensor_add(h2, h2, ln2_bt)
                    h2s.append((h2, slice(itl * P, (itl + 1) * P)))
                for h2, lsl in h2s:
                    pt2 = ps_tr1.tile([P, C], BF, tag="p1k")
                    for kc in range(NCC):
                        nc.tensor.transpose(pt2[:, kc * P:(kc + 1) * P],
                                            h2[:, kc * P:(kc + 1) * P],
                                            ident)
                    nc.scalar.copy(
                        out=h2T[:, :, lsl],
                        in_=pt2.rearrange("p (kc t) -> p kc t", kc=NCC))

            # ---------------- qkv/att scope --------------------------------
            mid = ExitStack()
            pool_qkv = mid.enter_context(tc.tile_pool(name="qkv", bufs=1))
            pool_att = mid.enter_context(tc.tile_pool(name="att", bufs=2))
            qT = pool_qkv.tile([P, HL // 2, T], BF)   # [(2h,64d), hp, T]
            kT = pool_qkv.tile([P, HL // 2, T], BF)
            vaug = pool_qkv.tile([P, NT, HL, D + 1], BF)

            # ------- prep (fused per-chunk): LN1 -> q/kv/RoPE --------------
            # One loop over 16 token chunks keeps the PE fed with matmuls
            # from the start (HAM stays warm) and lets DVE LN/RoPE for chunk
            # i+1 overlap PE work for chunk i. hT/ckvT chunks are only used
            # within their iteration, so they rotate in small pools.
            with ExitStack() as prep:
                pool_hT = prep.enter_context(tc.tile_pool(name="hT", bufs=3))
                pool_ckvT = prep.enter_context(tc.tile_pool(name="ckvT", bufs=3))
                pool_xsp = prep.enter_context(tc.tile_pool(name="xs", bufs=4))
                pool_stp = prep.enter_context(tc.tile_pool(name="st", bufs=6))
                pool_tokp = prep.enter_context(tc.tile_pool(name="tok", bufs=4))
                pool_w = prep.enter_context(tc.tile_pool(name="wts", bufs=1))
                pool_cs = prep.enter_context(tc.tile_pool(name="cs", bufs=4))
                pool_ro = prep.enter_context(tc.tile_pool(name="ro", bufs=6))
                # 4 accumulators per chunk: bufs=6 gives 1.5 chunks of
                # cross-chunk pipeline depth (the 3-slot ring serialized
                # chunks almost completely)
                ps_big = prep.enter_context(
                    tc.tile_pool(name="psbig", bufs=6, space="PSUM"))
                ps_tr = prep.enter_context(
                    tc.tile_pool(name="pstr", bufs=2, space="PSUM"))

                # x/cos loads for the first chunks are emitted BEFORE the
                # weight loads so the LN1 chain starts at ~2us instead of
                # queueing behind 4MB of weights on the same DMA queue
                xstash = {}

                def load_x(it):
                    sl = slice(it * P, (it + 1) * P)
                    xt = pool_xsp.tile([P, C], F32)
                    nc.sync.dma_start(out=xt, in_=x_d[sl, :])
                    cos_sb = pool_cs.tile([P, D], BF, tag="cos")
                    nc.sync.dma_start(out=cos_sb, in_=cos_d[sl, :])
                    sinf_sb = pool_cs.tile([P, D], BF, tag="sinf")
                    nc.sync.dma_start(out=sinf_sb, in_=sinf_d[sl, :])
                    xstash[it] = (xt, cos_sb, sinf_sb)

                load_x(0)
                load_x(1)
                kvdw_sb = load_chunks(pool_w, kvd_d, NCC, R, "kvdw")
                qw_sb = load_chunks(pool_w, qw_d, NCC, HD, "qw")
                kvuk_sb = load_chunks(pool_w, kvuk_d, NRC, HD, "kvuk")
                kvuv_sb = load_chunks(pool_w, kvuv_d, NRC, HD, "kvuv")

                def rope(ps, cos_sb, sinf_sb):
                    # ACT evicts psum to a flat bf16 tile (plain write, no
                    # view); all-bf16 DVE ops then run at 2x rate
                    qbf = pool_ro.tile([P, HD], BF, tag="qbf", name="qbf")
                    nc.scalar.copy(out=qbf, in_=ps)
                    psv = qbf.rearrange("p (h d) -> p h d", d=D)
                    t1 = pool_ro.tile([P, HL, D], BF, tag="t1")
                    nc.vector.tensor_mul(t1, psv, bcast_free(cos_sb, HL, D))
                    t2 = pool_ro.tile([P, HL, D], BF, tag="t2")
                    half = D // 2
                    sfv = sinf_sb
                    nc.vector.tensor_mul(
                        t2[:, :, 0:half],
                        bass.AP(tensor=psv.tensor, offset=psv.offset + half,
                                ap=[psv.ap[0], [D, HL], [1, half]]),
                        bass.AP(tensor=sfv.tensor, offset=sfv.offset,
                                ap=[sfv.ap[0], [0, HL], [1, half]]))
                    nc.vector.tensor_mul(
                        t2[:, :, half:D],
                        bass.AP(tensor=psv.tensor, offset=psv.offset,
                                ap=[psv.ap[0], [D, HL], [1, half]]),
                        bass.AP(tensor=sfv.tensor, offset=sfv.offset + half,
                                ap=[sfv.ap[0], [0, HL], [1, half]]))
                    ro = pool_ro.tile([P, HL, D], BF, tag="ro")
                    nc.vector.tensor_add(ro, t1, t2)
                    return ro.rearrange("p h d -> p (h d)")

                def evict_pairs(ro_flat, dstT, sl):
                    # [tok, (2 heads x 64d)] pair-chunks -> dstT[(par,d), hp]
                    for hp in range(HL // 2):
                        pt = ps_tr.tile([P, P], BF, tag="ptr")
                        nc.tensor.transpose(
                            pt, ro_flat[:, hp * P:(hp + 1) * P], ident)
                        if hp % 2 == 0:
                            nc.scalar.copy(out=dstT[:, hp, sl], in_=pt)
                        else:
                            nc.vector.tensor_copy(out=dstT[:, hp, sl], in_=pt)

                # Explicit two-stage software pipeline: stage A of chunk
                # i+1 is emitted before stage B of chunk i, so every engine
                # queue interleaves adjacent chunks (the scheduler's cost
                # model underestimates DVE latency and otherwise serializes
                # the whole per-chunk chain).
                stash = {}

                def stage_a(it):
                    sl = slice(it * P, (it + 1) * P)
                    if it not in xstash:
                        load_x(it)
                    xt, cos_sb, sinf_sb = xstash[it]

                    mv = ln_stats(pool_stp, xt, C, eps_t)
                    ht = pool_tokp.tile([P, C], BF, tag="ht")
                    nc.vector.tensor_scalar(out=ht, in0=xt,
                                            scalar1=mv[:, 0:1], scalar2=mv[:, 1:2],
                                            op0=ALU.subtract, op1=ALU.mult)
                    if ln1_wt is not None:
                        nc.vector.tensor_mul(ht, ht, ln1_wt)
                        nc.vector.tensor_add(ht, ht, ln1_bt)
                    hTc = pool_hT.tile([P, NCC, P], BF, tag="hTc")
                    for kc in range(NCC):
                        pt = ps_tr.tile([P, P], BF, tag="ptr")
                        nc.tensor.transpose(pt, ht[:, kc * P:(kc + 1) * P], ident)
                        if kc % 2 == 0:
                            nc.scalar.copy(out=hTc[:, kc, :], in_=pt)
                        else:
                            nc.vector.tensor_copy(out=hTc[:, kc, :], in_=pt)

                    psq = ps_big.tile([P, HD], F32, tag="psb")
                    for kc in range(NCC):
                        nc.tensor.matmul(psq, lhsT=hTc[:, kc, :],
                                         rhs=qw_sb[:, kc, :],
                                         start=(kc == 0), stop=(kc == NCC - 1))
                    ps = ps_big.tile([P, R], F32, tag="psb")
                    for kc in range(NCC):
                        nc.tensor.matmul(ps, lhsT=hTc[:, kc, :],
                                         rhs=kvdw_sb[:, kc, :],
                                         start=(kc == 0), stop=(kc == NCC - 1))
                    stash[it] = (psq, ps, cos_sb, sinf_sb)

                def stage_b(it):
                    sl = slice(it * P, (it + 1) * P)
                    psq, ps, cos_sb, sinf_sb = stash.pop(it)
                    ro_q = rope(psq, cos_sb, sinf_sb)

                    mv = ln_stats(pool_stp, ps, R, eps_t)
                    ct = pool_tokp.tile([P, R], BF, tag="ckvtok")
                    nc.vector.tensor_scalar(out=ct, in0=ps,
                                            scalar1=mv[:, 0:1], scalar2=mv[:, 1:2],
                                            op0=ALU.subtract, op1=ALU.mult)
                    if kvln_wt is not None:
                        nc.vector.tensor_mul(ct, ct, kvln_wt)
                        nc.vector.tensor_add(ct, ct, kvln_bt)

                    evict_pairs(ro_q, qT, sl)

                    ckvTc = pool_ckvT.tile([P, NRC, P], BF, tag="ckvTc")
                    for rc in range(NRC):
                        pt = ps_tr.tile([P, P], BF, tag="ptr")
                        nc.tensor.transpose(pt, ct[:, rc * P:(rc + 1) * P], ident)
                        if rc % 2 == 0:
                            nc.scalar.copy(out=ckvTc[:, rc, :], in_=pt)
                        else:
                            nc.vector.tensor_copy(out=ckvTc[:, rc, :], in_=pt)

                    psk = ps_big.tile([P, HD], F32, tag="psb")
                    for rc in range(NRC):
                        nc.tensor.matmul(psk, lhsT=ckvTc[:, rc, :],
                                         rhs=kvuk_sb[:, rc, :],
                                         start=(rc == 0), stop=(rc == NRC - 1))
                    ro_k = rope(psk, cos_sb, sinf_sb)

                    psv = ps_big.tile([P, HD], F32, tag="psb")
                    for rc in range(NRC):
                        nc.tensor.matmul(psv, lhsT=ckvTc[:, rc, :],
                                         rhs=kvuv_sb[:, rc, :],
                                         start=(rc == 0), stop=(rc == NRC - 1))

                    evict_pairs(ro_k, kT, sl)

                    nc.vector.memset(vaug[:, it, :, D:D + 1], 1.0)
                    nc.scalar.copy(out=vaug[:, it, :, 0:D],
                                   in_=psv.rearrange("p (h d) -> p h d", d=D))

                stage_a(0)
                for it in range(NT):
                    if it + 1 < NT:
                        stage_a(it + 1)
                    stage_b(it)

            # ---- Phase 3+5a+5b: attention / proj / residual interleaved ---
            with ExitStack() as attn:
                pool_p = attn.enter_context(tc.tile_pool(name="pexp", bufs=16))
                pool_s = attn.enter_context(tc.tile_pool(name="srow", bufs=4))
                pool_bc = attn.enter_context(tc.tile_pool(name="bc", bufs=4))
                pool_pw = attn.enter_context(tc.tile_pool(name="pw", bufs=1))
                pool_sa = attn.enter_context(tc.tile_pool(name="sa", bufs=3))
                ps_s = attn.enter_context(
                    tc.tile_pool(name="pss", bufs=4, space="PSUM"))
                ps_o = attn.enter_context(
                    tc.tile_pool(name="pso", bufs=1, space="PSUM"))
                ps_pj = attn.enter_context(
                    tc.tile_pool(name="pspj", bufs=1, space="PSUM"))
                pw_sb = load_chunks(pool_pw, pw_d, NRC, C, "pw")
                NPAIR = NT // 2
                att_cur = [None]  # per-slab [(2h,64d), hp, SLB] tile

                def proj_slab(n):
                    # token-major partial: out[tok, C] = att[:, hp, tok].T @ pw
                    att = att_cur[0]
                    for itl in range(SLB // P):
                        tsl = slice(itl * P, (itl + 1) * P)
                        ps = ps_pj.tile([P, C], F32, tag="pspj")
                        for half in range(2):
                            hsl = slice(half * 512, (half + 1) * 512)
                            for hp in range(NRC):
                                nc.tensor.matmul(ps[:, hsl],
                                                 lhsT=att[:, hp, tsl],
                                                 rhs=pw_sb[:, hp, hsl],
                                                 start=(hp == 0),
                                                 stop=(hp == NRC - 1))
                        sa_t = pool_sa.tile([P, C], BF, tag="sat")
                        if projb_t is not None:
                            nc.vector.tensor_add(sa_t, ps, projb_t)
                        else:
                            nc.scalar.copy(out=sa_t, in_=ps)
                        nc.sync.dma_start(out=cc1_in[n][tsl, :], in_=sa_t)
                    nc.gpsimd.collective_compute(
                        "AllReduce", ALU.add, replica_groups=groups,
                        ins=[cc1_in[n]], outs=[cc1_out[n]])

                # Per head-pair, interleave the two heads (PE row groups 0-63
                # and 64-127) and pipeline qk of group g+1 ahead of av of
                # group g so the Exp never stalls the PE.
                GRP = 2  # key-blocks per score group
                for pi in range(NPAIR):
                    if pi % 2 == 0:
                        att_cur[0] = pool_att.tile([P, NRC, SLB], BF,
                                                   tag="attsl", name="attsl")
                    q0 = 2 * pi            # first q-block of pair
                    qsl = slice(q0 * P, (q0 + 2) * P)     # 256 queries
                    asl = slice((pi % 2) * 2 * P, (pi % 2 + 1) * 2 * P)
                    nkb = 2 * pi + 2
                    ngr = nkb // GRP

                    for hp in range(HL // 2):
                        poA = ps_o.tile([P, 2 * P], F32, tag="poA", name="poA")
                        poB = ps_o.tile([P, 2 * P], F32, tag="poB", name="poB")
                        pos = {0: poA, 64: poB}

                        def qk_group(g, hr):
                            pss = ps_s.tile([P, GRP * 2 * P], F32, tag="pss")
                            for j in range(GRP):
                                kb = g * GRP + j
                                jsl = slice(j * 2 * P, (j + 1) * 2 * P)
                                nc.tensor.matmul(
                                    pss[:, jsl],
                                    lhsT=kT[hr:hr + 64, hp, kb * P:(kb + 1) * P],
                                    rhs=qT[hr:hr + 64, hp, qsl],
                                    start=True, stop=True)
                                if kb == q0:
                                    nc.vector.tensor_add(
                                        pss[:, j * 2 * P:j * 2 * P + P],
                                        pss[:, j * 2 * P:j * 2 * P + P], maskT)
                                elif kb == q0 + 1:
                                    nc.vector.tensor_add(
                                        pss[:, jsl], pss[:, jsl],
                                        mask2.rearrange("p a b -> p (a b)"))
                            pexp = pool_p.tile([P, GRP * 2 * P], BF, tag="pexp")
                            nc.scalar.activation(out=pexp, in_=pss,
                                                 func=AF.Exp,
                                                 scale=float(D) ** -0.5)
                            return pexp

                        def av_group(g, hr, pexp):
                            po = pos[hr]
                            h = 2 * hp + (1 if hr else 0)
                            for j in range(GRP):
                                kb = g * GRP + j
                                nc.tensor.matmul(
                                    po[0:D + 1, :],
                                    lhsT=vaug[:, kb, h, :],
                                    rhs=pexp[:, j * 2 * P:(j + 1) * 2 * P],
                                    start=(kb == 0), stop=(kb == nkb - 1))

                        # all qk groups first (64-row array mode, the two
                        # heads in disjoint row groups), then all av groups
                        # (128-row mode): two mode switches per head-pair
                        # instead of two per group
                        pexps = []
                        for g in range(ngr):
                            pexps.append((qk_group(g, 0), qk_group(g, 64)))
                        for g in range(ngr):
                            av_group(g, 0, pexps[g][0])
                            av_group(g, 64, pexps[g][1])

                        # normalization fused into eviction
                        for hr in (0, 64):
                            po = pos[hr]
                            srow = pool_s.tile([1, 2 * P], F32, tag="srow")
                            nc.vector.tensor_copy(out=srow, in_=po[D:D + 1, :])
                            bc = pool_bc.tile([64, 2 * P], F32, tag="bc")
                            nc.gpsimd.partition_broadcast(out_ap=bc, in_ap=srow)
                            rc_ = pool_bc.tile([64, 2 * P], F32, tag="rc")
                            nc.vector.reciprocal_approx_fast(out=rc_, in_=bc)
                            nc.vector.tensor_mul(
                                att_cur[0][hr:hr + 64, hp, asl],
                                po[0:D, :], rc_)
                    if pi % 2 == 1:
                        proj_slab(pi // 2)

                # gate: depends on the last pair's attention eviction
                nc.vector.tensor_copy(out=gate_t,
                                      in_=att_cur[0][0:1, NRC - 1,
                                                     SLB - 4:SLB])

            mid.close()  # release qT/kT/vaug/att SBUF

            # ------- Back half: FFN + second AllReduce + final -------------
            with ExitStack() as bh:
                pool_fw = bh.enter_context(tc.tile_pool(name="fw", bufs=1))
                pool_g = bh.enter_context(tc.tile_pool(name="gT", bufs=2))
                pool_ev = bh.enter_context(tc.tile_pool(name="ev", bufs=3))
                pool_ffin = bh.enter_context(tc.tile_pool(name="ffin", bufs=3))
                pool_of = bh.enter_context(tc.tile_pool(name="of", bufs=3))
                ps_f = bh.enter_context(
                    tc.tile_pool(name="psf", bufs=2, space="PSUM"))
                ps_f2 = bh.enter_context(
                    tc.tile_pool(name="psf2", bufs=2, space="PSUM"))
                ps_tr1 = bh.enter_context(
                    tc.tile_pool(name="pstr1", bufs=2, space="PSUM"))
                fivb_pools["sa_in"] = bh.enter_context(
                    tc.tile_pool(name="sain", bufs=2))
                fivb_pools["x2"] = bh.enter_context(
                    tc.tile_pool(name="x2", bufs=2))
                fivb_pools["st2"] = bh.enter_context(
                    tc.tile_pool(name="st2", bufs=4))
                fivb_pools["tok2"] = bh.enter_context(
                    tc.tile_pool(name="tok2", bufs=4))

                # f1/f2 weight loads first: the DMAs overlap the attention
                # tail (fresh pool space, no released-zone wait on hot tiles)
                f1w_sb = load_chunks(pool_fw, f1_d, NCC, FH, "f1w")
                f2w_sb = load_chunks(pool_fw, f2_d, FH // P, C, "f2w")

                gT_sl = [None] * NSL
                NIT = SLB // P  # 4 token blocks per slab

                def f1_slab(n):
                    h2T = h2T_sl[n]
                    gT = pool_g.tile([P, FH // P, SLB], BF, tag="gT")
                    gT_sl[n] = gT
                    for m in range(FH // P):
                        ps = ps_f.tile([P, SLB], F32, tag="psf1")
                        for kc in range(NCC):
                            nc.tensor.matmul(ps,
                                             lhsT=f1w_sb[:, kc, m * P:(m + 1) * P],
                                             rhs=h2T[:, kc, :],
                                             start=(kc == 0),
                                             stop=(kc == NCC - 1))
                        if f1b_t is not None:
                            nc.scalar.activation(out=gT[:, m, :], in_=ps,
                                                 func=AF.Gelu,
                                                 bias=f1b_t[:, m:m + 1],
                                                 scale=1.0)
                        else:
                            nc.scalar.activation(out=gT[:, m, :], in_=ps,
                                                 func=AF.Gelu, scale=1.0)

                def cc2_home(it):
                    t = it * P
                    for ci, (t0, ln) in enumerate(CC2):
                        if t0 <= t < t0 + ln:
                            return ci, t - t0
                    raise AssertionError(it)

                def f2_block(it):
                    """f2 partial for global 128-token block it, token-major."""
                    n, itl = it // NIT, it % NIT
                    gT = gT_sl[n]
                    tsl = slice(itl * P, (itl + 1) * P)
                    ps = ps_f2.tile([P, C], F32, tag="psf2")
                    for half in range(2):
                        hsl = slice(half * 512, (half + 1) * 512)
                        for kf in range(FH // P):
                            nc.tensor.matmul(ps[:, hsl],
                                             lhsT=gT[:, kf, tsl],
                                             rhs=f2w_sb[:, kf, hsl],
                                             start=(kf == 0),
                                             stop=(kf == FH // P - 1))
                    ev = pool_ev.tile([P, C], BF, tag="ffev")
                    if f2b_t is not None:
                        nc.vector.tensor_add(ev, ps, f2b_t)
                    else:
                        nc.scalar.copy(out=ev, in_=ps)
                    ci, row = cc2_home(it)
                    nc.sync.dma_start(out=cc2_in[ci][row:row + P, :], in_=ev)

                def cc2_launch(ci):
                    nc.gpsimd.collective_compute(
                        "AllReduce", ALU.add, replica_groups=groups,
                        ins=[cc2_in[ci]], outs=[cc2_out[ci]])

                def emit_final(ci):
                    """out = x + sa + ff for collective chunk ci (no PE)."""
                    t0, ln = CC2[ci]
                    for j in range(ln // P):
                        itg = t0 // P + j
                        sl = slice(itg * P, (itg + 1) * P)
                        xt = pool_xs.tile([P, C], F32, tag="xt2")
                        nc.sync.dma_start(out=xt, in_=x_d[sl, :])
                        saf = pool_ffin.tile([P, C], BF, tag="sfin")
                        nc.vector.tensor_copy(out=saf[0:1, 0:4], in_=gate_t)
                        nc.sync.dma_start(
                            out=saf,
                            in_=cc1_out[itg // NIT][(itg % NIT) * P:
                                                    (itg % NIT + 1) * P, :])
                        ff = pool_ffin.tile([P, C], BF, tag="ffin")
                        nc.vector.tensor_copy(out=ff[0:1, 0:4], in_=gate_t)
                        nc.sync.dma_start(out=ff,
                                          in_=cc2_out[ci][j * P:(j + 1) * P, :])
                        ot = pool_of.tile([P, C], F32, tag="of")
                        nc.vector.tensor_add(ot, xt, saf)
                        nc.gpsimd.tensor_add(ot, ot, ff)
                        nc.sync.dma_start(out=out_d[sl, :], in_=ot)

                emit_5b(0)
                f1_slab(0)
                emit_5b(1)
                for it in range(0, 4):
                    f2_block(it)
                cc2_launch(0)
                f1_slab(1)
                emit_5b(2)
                for it in range(4, 8):
                    f2_block(it)
                cc2_launch(1)
                emit_final(0)
                f1_slab(2)
                emit_5b(3)
                for it in range(8, 12):
                    f2_block(it)
                cc2_launch(2)
                emit_final(1)
                f1_slab(3)
                f2_block(12)
                f2_block(13)
                f2_block(14)
                cc2_launch(3)
                f2_block(15)
                cc2_launch(4)
                emit_final(2)
                emit_final(3)
                emit_final(4)

    nc.compile()
    return nc


def kernel(**inputs):
    x = np.asarray(inputs["x"], dtype=np.float32)
    q_w = np.asarray(inputs["q_w"], dtype=np.float32)
    kvd_w = np.asarray(inputs["kvd_w"], dtype=np.float32)
    kvu_w = np.asarray(inputs["kvu_w"], dtype=np.float32)
    proj_w = np.asarray(inputs["proj_w"], dtype=np.float32)
    f1_w = np.asarray(inputs["f1_w"], dtype=np.float32)
    f2_w = np.asarray(inputs["f2_w"], dtype=np.float32)
    ln1_w = np.asarray(inputs["ln1_w"], dtype=np.float32)
    ln1_b = np.asarray(inputs["ln1_b"], dtype=np.float32)
    ln2_w = np.asarray(inputs["ln2_w"], dtype=np.float32)
    ln2_b = np.asarray(inputs["ln2_b"], dtype=np.float32)
    kvln_w = np.asarray(inputs["kvln_w"], dtype=np.float32)
    kvln_b = np.asarray(inputs["kvln_b"], dtype=np.float32)
    proj_b = np.asarray(inputs["proj_b"], dtype=np.float32)
    f1_b = np.asarray(inputs["f1_b"], dtype=np.float32)
    f2_b = np.asarray(inputs["f2_b"], dtype=np.float32)

    flags = (bool(np.allclose(ln1_w, 1) and np.allclose(ln1_b, 0)),
             bool(np.allclose(kvln_w, 1) and np.allclose(kvln_b, 0)),
             bool(np.allclose(ln2_w, 1) and np.allclose(ln2_b, 0)),
             bool(np.allclose(proj_b, 0)),
             bool(np.allclose(f1_b, 0)),
             bool(np.allclose(f2_b, 0)))
    if flags not in _CACHE:
        _CACHE[flags] = _build(flags)
    nc = _CACHE[flags]

    cos, sinf = _rope_tables()
    kvu_v4 = kvu_w.reshape(R, 2, H, D)
    in_maps = []
    for c in range(8):
        b, half = c // 2, c % 2
        hsl = slice(half * HL, (half + 1) * HL)
        m = {
            "x_loc": np.ascontiguousarray(x[b]),
            "qw_loc": np.ascontiguousarray(
                q_w[:, half * HD:(half + 1) * HD]).astype(BF16),
            "kvd_w": kvd_w.astype(BF16),
            "kvu_k": np.ascontiguousarray(
                kvu_v4[:, 0, hsl, :].reshape(R, HD)).astype(BF16),
            "kvu_v": np.ascontiguousarray(
                kvu_v4[:, 1, hsl, :].reshape(R, HD)).astype(BF16),
            "proj_w_loc": np.ascontiguousarray(
                proj_w[half * HD:(half + 1) * HD, :]).astype(BF16),
            "f1_w_loc": np.ascontiguousarray(
                f1_w[:, half * FH:(half + 1) * FH]).astype(BF16),
            "f2_w_loc": np.ascontiguousarray(
                f2_w[half * FH:(half + 1) * FH, :]).astype(BF16),
            "cos_t": cos.astype(BF16),
            "sinf_t": sinf.astype(BF16),
        }
        if not flags[0]:
            m["ln1_w"], m["ln1_b"] = ln1_w, ln1_b
        if not flags[1]:
            m["kvln_w"], m["kvln_b"] = kvln_w, kvln_b
        if not flags[2]:
            m["ln2_w"], m["ln2_b"] = ln2_w, ln2_b
        if not flags[3]:
            m["proj_b"] = proj_b
        if not flags[4]:
            m["f1_b_loc"] = np.ascontiguousarray(f1_b[half * FH:(half + 1) * FH])
        if not flags[5]:
            m["f2_b"] = f2_b
        in_maps.append(m)

    res = run_bass_kernel_spmd(nc, in_maps, list(range(8)), trace=TRACE)
    kernel.last_result = res
    out = np.stack([res.results[2 * b]["out_loc"] for b in range(B)])
    return out



# revision 61
# speedup vs baseline: 1.1741x; 1.0029x over previous
"""Trainium2 Bass kernel for nn_Block_11897059410591 (MLA transformer block).

Sharding over 8 NeuronCores: core c = (batch b=c//2, head-half h0=(c%2)*8).
Each core computes LN1/kvd/kvu/RoPE for its whole batch, causal attention for
its 8 heads, a partial output projection (contracted over its heads) that is
pair-AllReduced, then the FFN with d_ff split in half across the pair and a
second pair-AllReduce. Both cores of a pair end with the identical full-batch
output; the host keeps the even core's copy.

proj and f2 emit token-major partials so the collective payloads are
[tokens, C] and the residual/LN2/final paths need no PE transposes; the final
output recomputes x + sa + ff from DRAM so nothing is buffered across the
back half. The trailing f2 collective is split 512/512/512/384/128 so the
exposed tail is one 0.25 MB AllReduce.

Scheduling notes (the Tile scheduler orders each engine queue by its own
simulated readiness, so structure must be encoded in emission order + deps):
- a tiny warm-up AllReduce at t=0 absorbs the CC stack's cold start;
- prep is emitted as an explicit two-stage software pipeline (stage A of
  chunk i+1 before stage B of chunk i) so adjacent chunks overlap;
- RoPE runs in all-bf16 on DVE (2x rate) off a flat ACT-evicted psum copy;
- attention emits all qk score groups (64-row PE mode, head pair in
  disjoint row groups) before all av groups (128-row mode) per head-pair;
- back-half tiles whose loads depend on collective outputs take a 1-element
  probe-write reading a gate produced by the last attention eviction, so
  their loads cannot be hoisted into the attention region of the queues
  where a slow AllReduce would head-of-line-block everything behind it.
"""
import sys

if "/opt/trn_rl_repo" not in sys.path:
    sys.path.insert(0, "/opt/trn_rl_repo")

import numpy as np
import ml_dtypes


def _ensure_ntff_hook():
    """antenv.axon_hooks is missing in this image; shim it so
    run_bass_kernel_spmd(trace=True) can capture NTFF profiles."""
    try:
        from antenv import axon_hooks  # noqa: F401
        return
    except ImportError:
        pass
    try:
        import types
        import importlib.util
        m = types.ModuleType("antenv.axon_hooks")
        _hook = [None]
        m.set_axon_ntff_profile_hook = lambda h: _hook.__setitem__(0, h)
        m.get_axon_ntff_profile_hook = lambda: _hook[0]
        sys.modules["antenv.axon_hooks"] = m
        import antenv
        antenv.axon_hooks = m
        spec = importlib.util.spec_from_file_location(
            "_trn_boot_shim", "/root/.axon_site/trn_agent_boot/trn_boot.py")
        tb = importlib.util.module_from_spec(spec)
        spec.loader.exec_module(tb)
        hook = tb._ntff_profile_via_ctypes("/opt/axon/libaxon_pjrt.so")
        m.set_axon_ntff_profile_hook(hook)
    except Exception as e:  # degrade to trace-less operation
        print(f"ntff hook shim failed ({e}); tracing disabled", file=sys.stderr)


_ensure_ntff_hook()

import concourse.bass as bass
import concourse.mybir as mybir
import concourse.tile as tile
from concourse import bacc
from concourse.bass_utils import run_bass_kernel_spmd
from concourse.masks import make_identity

F32 = mybir.dt.float32
BF = mybir.dt.bfloat16
BF16 = ml_dtypes.bfloat16
AF = mybir.ActivationFunctionType
ALU = mybir.AluOpType

B, T, C = 4, 2048, 1024
H, D, R, FF = 16, 64, 512, 4096
HL = 8              # heads per core
HD = HL * D         # 512
FH = FF // 2        # 2048, d_ff half per core
P = 128
NT = T // P         # 16 token chunks
NCC = C // P        # 8 C chunks
NRC = R // P        # 4 R chunks
NSL = 4             # token slabs for the back half
SLB = T // NSL      # 512 tokens per slab
LN_EPS = 1e-5

TRACE = False
_CACHE = {}


def _rope_tables():
    inv_freq = 1.0 / (10000.0 ** (np.arange(0, D, 2, dtype=np.float32) / D))
    t = np.arange(T, dtype=np.float32)
    freqs = np.outer(t, inv_freq)
    emb = np.concatenate([freqs, freqs], axis=-1)  # [T, D]
    cos = np.cos(emb).astype(np.float32)
    sin = np.sin(emb).astype(np.float32)
    sinf = sin.copy()
    sinf[:, : D // 2] = -sinf[:, : D // 2]
    return cos, sinf


def _build(flags):
    (ln1_triv, kvln_triv, ln2_triv, pb0, f1b0, f2b0) = flags
    nc = bacc.Bacc("TRN2", target_bir_lowering=False, debug=False,
                   enable_asserts=False, num_devices=8)

    x_d = nc.dram_tensor("x_loc", [T, C], F32, kind="ExternalInput").ap()
    qw_d = nc.dram_tensor("qw_loc", [C, HD], BF, kind="ExternalInput").ap()
    kvd_d = nc.dram_tensor("kvd_w", [C, R], BF, kind="ExternalInput").ap()
    kvuk_d = nc.dram_tensor("kvu_k", [R, HD], BF, kind="ExternalInput").ap()
    kvuv_d = nc.dram_tensor("kvu_v", [R, HD], BF, kind="ExternalInput").ap()
    pw_d = nc.dram_tensor("proj_w_loc", [HD, C], BF, kind="ExternalInput").ap()
    f1_d = nc.dram_tensor("f1_w_loc", [C, FH], BF, kind="ExternalInput").ap()
    f2_d = nc.dram_tensor("f2_w_loc", [FH, C], BF, kind="ExternalInput").ap()
    cos_d = nc.dram_tensor("cos_t", [T, D], BF, kind="ExternalInput").ap()
    sinf_d = nc.dram_tensor("sinf_t", [T, D], BF, kind="ExternalInput").ap()
    out_d = nc.dram_tensor("out_loc", [T, C], F32, kind="ExternalOutput").ap()

    opt_ins = {}
    if not ln1_triv:
        opt_ins["ln1_w"] = nc.dram_tensor("ln1_w", [C], F32, kind="ExternalInput").ap()
        opt_ins["ln1_b"] = nc.dram_tensor("ln1_b", [C], F32, kind="ExternalInput").ap()
    if not kvln_triv:
        opt_ins["kvln_w"] = nc.dram_tensor("kvln_w", [R], F32, kind="ExternalInput").ap()
        opt_ins["kvln_b"] = nc.dram_tensor("kvln_b", [R], F32, kind="ExternalInput").ap()
    if not ln2_triv:
        opt_ins["ln2_w"] = nc.dram_tensor("ln2_w", [C], F32, kind="ExternalInput").ap()
        opt_ins["ln2_b"] = nc.dram_tensor("ln2_b", [C], F32, kind="ExternalInput").ap()
    if not pb0:
        opt_ins["proj_b"] = nc.dram_tensor("proj_b", [C], F32, kind="ExternalInput").ap()
    if not f1b0:
        opt_ins["f1_b"] = nc.dram_tensor("f1_b_loc", [FH], F32, kind="ExternalInput").ap()
    if not f2b0:
        opt_ins["f2_b"] = nc.dram_tensor("f2_b", [C], F32, kind="ExternalInput").ap()

    # internal DRAM (all collective payloads token-major)
    cc1_in = [nc.dram_tensor(f"cc1_in{n}", [SLB, C], BF).ap()
              for n in range(NSL)]
    cc1_out = [nc.dram_tensor(f"cc1_out{n}", [SLB, C], BF).ap()
               for n in range(NSL)]
    ccw_in = nc.dram_tensor("ccw_in", [P, 2], F32).ap()
    ccw_out = nc.dram_tensor("ccw_out", [P, 2], F32).ap()
    # f2 collective chunks: (token_start, n_tokens); only the last chunk is
    # small (per-op fixed cost ~10us dominates below ~0.5MB, so an evenly
    # fine split would serialize on the CC stream)
    CC2 = [(0, 512), (512, 512), (1024, 512), (1536, 384), (1920, 128)]
    cc2_in = [nc.dram_tensor(f"cc2_in{ci}", [ln, C], BF).ap()
              for ci, (t0, ln) in enumerate(CC2)]
    cc2_out = [nc.dram_tensor(f"cc2_out{ci}", [ln, C], BF).ap()
               for ci, (t0, ln) in enumerate(CC2)]
    groups = [[0, 1], [2, 3], [4, 5], [6, 7]]

    def bcast_free(ap2d, n, width):
        """[P, width] AP -> [P, n, width] with 0-step middle dim."""
        return bass.AP(tensor=ap2d.tensor, offset=ap2d.offset,
                       ap=[ap2d.ap[0], [0, n], [1, width]])

    def ln_stats(pool, src_ap, width, eps_t):
        """Per-partition (mean, rstd) of src_ap [P, width]."""
        ngr = (width + 511) // 512
        st6 = pool.tile([P, ngr, 6], F32, tag="st6")
        sv = src_ap.rearrange("p (g d) -> p g d", g=ngr)
        for g in range(ngr):
            nc.vector.bn_stats(out=st6[:, g, :], in_=sv[:, g, :])
        mv = pool.tile([P, 2], F32, tag="mv")
        nc.vector.bn_aggr(out=mv, in_=st6)
        nc.scalar.activation(out=mv[:, 1:2], in_=mv[:, 1:2], func=AF.Sqrt,
                             bias=eps_t, scale=1.0)
        nc.vector.reciprocal(out=mv[:, 1:2], in_=mv[:, 1:2])
        return mv

    from contextlib import ExitStack
    with tile.TileContext(nc) as tc:
        with ExitStack() as ctx:
            const = ctx.enter_context(tc.tile_pool(name="const", bufs=1))
            ident = const.tile([P, P], BF)
            make_identity(nc, ident)
            eps_t = const.tile([P, 1], F32)
            nc.vector.memset(eps_t, LN_EPS)
            # S^T diagonal causal mask: keep (0) where col(q) >= row(k)
            maskT = const.tile([P, P], F32)
            nc.gpsimd.memset(maskT, 0.0)
            nc.gpsimd.affine_select(out=maskT, in_=maskT, compare_op=ALU.is_ge,
                                    fill=-1e9, base=0, pattern=[[1, P]],
                                    channel_multiplier=-1)
            mask_full = const.tile([P, P], F32)
            nc.vector.memset(mask_full, -1e9)
            # Written at the end of attention; back-half tiles that depend on
            # collective outputs take a 1-element probe-write reading this
            # first, so the scheduler cannot hoist their loads (and the DVE
            # ops behind them) into the attention region of the engine
            # queues, where a slow AllReduce would head-of-line-block it.
            gate_t = const.tile([1, 4], F32, name="gate")
            # [full | diag] combined mask for the kb==q0+1 block pair
            mask2 = const.tile([P, 2, P], F32)
            nc.vector.memset(mask2[:, 0, :], -1e9)
            nc.vector.tensor_copy(out=mask2[:, 1, :], in_=maskT)

            # CC-stream warm-up: a tiny AllReduce issued at t=0 absorbs the
            # collective stack's cold-start cost under the prep phase, so
            # the first real AllReduce runs at steady-state speed.
            ccw_t = const.tile([P, 2], F32)
            nc.vector.memset(ccw_t, 1.0)
            nc.sync.dma_start(out=ccw_in, in_=ccw_t)
            nc.gpsimd.collective_compute(
                "AllReduce", ALU.add, replica_groups=groups,
                ins=[ccw_in], outs=[ccw_out])

            def dram_row_bcast(name, ap1d, width):
                t = const.tile([P, width], F32, name=name)
                src = bass.AP(tensor=ap1d.tensor, offset=ap1d.offset,
                              ap=[[0, P], [1, width]])
                nc.sync.dma_start(out=t, in_=src)
                return t

            ln1_wt = ln1_bt = ln2_wt = ln2_bt = kvln_wt = kvln_bt = None
            if not ln1_triv:
                ln1_wt = dram_row_bcast("ln1w_b", opt_ins["ln1_w"], C)
                ln1_bt = dram_row_bcast("ln1b_b", opt_ins["ln1_b"], C)
            if not kvln_triv:
                kvln_wt = dram_row_bcast("kvlnw_b", opt_ins["kvln_w"], R)
                kvln_bt = dram_row_bcast("kvlnb_b", opt_ins["kvln_b"], R)
            if not ln2_triv:
                ln2_wt = dram_row_bcast("ln2w_b", opt_ins["ln2_w"], C)
                ln2_bt = dram_row_bcast("ln2b_b", opt_ins["ln2_b"], C)
            projb_t = f1b_t = f2b_t = None
            if not pb0:
                # token-major proj output -> bias is a broadcast row [*, C]
                projb_t = dram_row_bcast("projb_b", opt_ins["proj_b"], C)
            if not f1b0:
                f1b_t = const.tile([P, FH // P], F32, name="f1b")
                nc.sync.dma_start(out=f1b_t, in_=opt_ins["f1_b"].rearrange(
                    "(m p) -> p m", p=P))
            if not f2b0:
                f2b_t = dram_row_bcast("f2b_b", opt_ins["f2_b"], C)

            def load_chunks(pool, dram_ap, nk, width, name):
                t = pool.tile([P, nk, width], BF, name=name)
                for k in range(nk):
                    nc.sync.dma_start(out=t[:, k, :],
                                      in_=dram_ap[k * P:(k + 1) * P, :])
                return t

            # ---- long-lived pools (created early; closed last, LIFO) ------
            pool_xs = ctx.enter_context(tc.tile_pool(name="xs2", bufs=3))
            pool_h2T = ctx.enter_context(tc.tile_pool(name="h2T", bufs=2))
            h2T_sl = [None] * NSL
            fivb_pools = {}

            def emit_5b(n):
                """Residual + LN2 + h2T for token slab n. cc1_out is
                token-major so no PE transposes are needed before LN2; only
                h2 -> h2T is transposed (for the f1 matmul rhs), in a second
                pass so the PE doesn't stall on the DVE LN chain."""
                h2T = pool_h2T.tile([P, NCC, SLB], BF, tag="h2T")
                h2T_sl[n] = h2T
                sa_in = fivb_pools["sa_in"].tile([P, SLB // P, C], BF,
                                                 tag="sa_in", name="sa_in")
                # probe-write: orders the loads after attention (see gate_t)
                nc.vector.tensor_copy(out=sa_in[0:1, 0, 0:4], in_=gate_t)
                for itl in range(SLB // P):
                    nc.sync.dma_start(
                        out=sa_in[:, itl, :],
                        in_=cc1_out[n][itl * P:(itl + 1) * P, :])
                h2s = []
                for itl in range(SLB // P):
                    it = n * (SLB // P) + itl
                    sl = slice(it * P, (it + 1) * P)
                    xt = pool_xs.tile([P, C], F32, tag="xt2")
                    nc.sync.dma_start(out=xt, in_=x_d[sl, :])
                    x2t = fivb_pools["x2"].tile([P, C], F32, tag="x2t",
                                                name="x2t")
                    # gpsimd is idle in the back half; freeing DVE here lets
                    # the LN2 chain (which feeds h2T -> f1) start sooner
                    nc.gpsimd.tensor_add(x2t, xt, sa_in[:, itl, :])
                    mv = ln_stats(fivb_pools["st2"], x2t, C, eps_t)
                    h2 = fivb_pools["tok2"].tile([P, C], BF, tag="h2", name="h2")
                    nc.vector.tensor_scalar(out=h2, in0=x2t,
                                            scalar1=mv[:, 0:1],
                                            scalar2=mv[:, 1:2],
                                            op0=ALU.subtract, op1=ALU.mult)
                    if ln2_wt is not None:
                        nc.vector.tensor_mul(h2, h2, ln2_wt)
                        nc.vector.tensor_add(h2, h2, ln2_bt)
                    h2s.append((h2, slice(itl * P, (itl + 1) * P)))
                for h2, lsl in h2s:
                    pt2 = ps_tr1.tile([P, C], BF, tag="p1k")
                    for kc in range(NCC):
                        nc.tensor.transpose(pt2[:, kc * P:(kc + 1) * P],
                                            h2[:, kc * P:(kc + 1) * P],
                                            ident)
                    nc.scalar.copy(
                        out=h2T[:, :, lsl],
                        in_=pt2.rearrange("p (kc t) -> p kc t", kc=NCC))

            # ---------------- qkv/att scope --------------------------------
            mid = ExitStack()
            pool_qkv = mid.enter_context(tc.tile_pool(name="qkv", bufs=1))
            pool_att = mid.enter_context(tc.tile_pool(name="att", bufs=2))
            qT = pool_qkv.tile([P, HL // 2, T], BF)   # [(2h,64d), hp, T]
            kT = pool_qkv.tile([P, HL // 2, T], BF)
            vaug = pool_qkv.tile([P, NT, HL, D + 1], BF)

            # ------- prep (fused per-chunk): LN1 -> q/kv/RoPE --------------
            # One loop over 16 token chunks keeps the PE fed with matmuls
            # from the start (HAM stays warm) and lets DVE LN/RoPE for chunk
            # i+1 overlap PE work for chunk i. hT/ckvT chunks are only used
            # within their iteration, so they rotate in small pools.
            with ExitStack() as prep:
                pool_hT = prep.enter_context(tc.tile_pool(name="hT", bufs=3))
                pool_ckvT = prep.enter_context(tc.tile_pool(name="ckvT", bufs=3))
                pool_xsp = prep.enter_context(tc.tile_pool(name="xs", bufs=4))
                pool_stp = prep.enter_context(tc.tile_pool(name="st", bufs=6))
                pool_tokp = prep.enter_context(tc.tile_pool(name="tok", bufs=4))
                pool_w = prep.enter_context(tc.tile_pool(name="wts", bufs=1))
                pool_cs = prep.enter_context(tc.tile_pool(name="cs", bufs=4))
                pool_ro = prep.enter_context(tc.tile_pool(name="ro", bufs=6))
                # 4 accumulators per chunk: bufs=6 gives 1.5 chunks of
                # cross-chunk pipeline depth (the 3-slot ring serialized
                # chunks almost completely)
                ps_big = prep.enter_context(
                    tc.tile_pool(name="psbig", bufs=6, space="PSUM"))
                ps_tr = prep.enter_context(
                    tc.tile_pool(name="pstr", bufs=2, space="PSUM"))

                # x/cos loads for the first chunks are emitted BEFORE the
                # weight loads so the LN1 chain starts at ~2us instead of
                # queueing behind 4MB of weights on the same DMA queue
                xstash = {}

                def load_x(it):
                    sl = slice(it * P, (it + 1) * P)
                    xt = pool_xsp.tile([P, C], F32)
                    nc.sync.dma_start(out=xt, in_=x_d[sl, :])
                    cos_sb = pool_cs.tile([P, D], BF, tag="cos")
                    nc.sync.dma_start(out=cos_sb, in_=cos_d[sl, :])
                    sinf_sb = pool_cs.tile([P, D], BF, tag="sinf")
                    nc.sync.dma_start(out=sinf_sb, in_=sinf_d[sl, :])
                    xstash[it] = (xt, cos_sb, sinf_sb)

                load_x(0)
                load_x(1)
                kvdw_sb = load_chunks(pool_w, kvd_d, NCC, R, "kvdw")
                qw_sb = load_chunks(pool_w, qw_d, NCC, HD, "qw")
                kvuk_sb = load_chunks(pool_w, kvuk_d, NRC, HD, "kvuk")
                kvuv_sb = load_chunks(pool_w, kvuv_d, NRC, HD, "kvuv")

                def rope(ps, cos_sb, sinf_sb):
                    # ACT evicts psum to a flat bf16 tile (plain write, no
                    # view); all-bf16 DVE ops then run at 2x rate
                    qbf = pool_ro.tile([P, HD], BF, tag="qbf", name="qbf")
                    nc.scalar.copy(out=qbf, in_=ps)
                    psv = qbf.rearrange("p (h d) -> p h d", d=D)
                    t1 = pool_ro.tile([P, HL, D], BF, tag="t1")
                    nc.vector.tensor_mul(t1, psv, bcast_free(cos_sb, HL, D))
                    t2 = pool_ro.tile([P, HL, D], BF, tag="t2")
                    half = D // 2
                    sfv = sinf_sb
                    nc.vector.tensor_mul(
                        t2[:, :, 0:half],
                        bass.AP(tensor=psv.tensor, offset=psv.offset + half,
                                ap=[psv.ap[0], [D, HL], [1, half]]),
                        bass.AP(tensor=sfv.tensor, offset=sfv.offset,
                                ap=[sfv.ap[0], [0, HL], [1, half]]))
                    nc.vector.tensor_mul(
                        t2[:, :, half:D],
                        bass.AP(tensor=psv.tensor, offset=psv.offset,
                                ap=[psv.ap[0], [D, HL], [1, half]]),
                        bass.AP(tensor=sfv.tensor, offset=sfv.offset + half,
                                ap=[sfv.ap[0], [0, HL], [1, half]]))
                    ro = pool_ro.tile([P, HL, D], BF, tag="ro")
                    nc.vector.tensor_add(ro, t1, t2)
                    return ro.rearrange("p h d -> p (h d)")

                def evict_pairs(ro_flat, dstT, sl):
                    # [tok, (2 heads x 64d)] pair-chunks -> dstT[(par,d), hp]
                    for hp in range(HL // 2):
                        pt = ps_tr.tile([P, P], BF, tag="ptr")
                        nc.tensor.transpose(
                            pt, ro_flat[:, hp * P:(hp + 1) * P], ident)
                        if hp % 2 == 0:
                            nc.scalar.copy(out=dstT[:, hp, sl], in_=pt)
                        else:
                            nc.vector.tensor_copy(out=dstT[:, hp, sl], in_=pt)

                # Explicit two-stage software pipeline: stage A of chunk
                # i+1 is emitted before stage B of chunk i, so every engine
                # queue interleaves adjacent chunks (the scheduler's cost
                # model underestimates DVE latency and otherwise serializes
                # the whole per-chunk chain).
                stash = {}

                def stage_a(it):
                    sl = slice(it * P, (it + 1) * P)
                    if it not in xstash:
                        load_x(it)
                    xt, cos_sb, sinf_sb = xstash[it]

                    mv = ln_stats(pool_stp, xt, C, eps_t)
                    ht = pool_tokp.tile([P, C], BF, tag="ht")
                    nc.vector.tensor_scalar(out=ht, in0=xt,
                                            scalar1=mv[:, 0:1], scalar2=mv[:, 1:2],
                                            op0=ALU.subtract, op1=ALU.mult)
                    if ln1_wt is not None:
                        nc.vector.tensor_mul(ht, ht, ln1_wt)
                        nc.vector.tensor_add(ht, ht, ln1_bt)
                    hTc = pool_hT.tile([P, NCC, P], BF, tag="hTc")
                    for kc in range(NCC):
                        pt = ps_tr.tile([P, P], BF, tag="ptr")
                        nc.tensor.transpose(pt, ht[:, kc * P:(kc + 1) * P], ident)
                        if kc % 2 == 0:
                            nc.scalar.copy(out=hTc[:, kc, :], in_=pt)
                        else:
                            nc.vector.tensor_copy(out=hTc[:, kc, :], in_=pt)

                    psq = ps_big.tile([P, HD], F32, tag="psb")
                    for kc in range(NCC):
                        nc.tensor.matmul(psq, lhsT=hTc[:, kc, :],
                                         rhs=qw_sb[:, kc, :],
                                         start=(kc == 0), stop=(kc == NCC - 1))
                    ps = ps_big.tile([P, R], F32, tag="psb")
                    for kc in range(NCC):
                        nc.tensor.matmul(ps, lhsT=hTc[:, kc, :],
                                         rhs=kvdw_sb[:, kc, :],
                                         start=(kc == 0), stop=(kc == NCC - 1))
                    stash[it] = (psq, ps, cos_sb, sinf_sb)

                def stage_b(it):
                    sl = slice(it * P, (it + 1) * P)
                    psq, ps, cos_sb, sinf_sb = stash.pop(it)
                    ro_q = rope(psq, cos_sb, sinf_sb)

                    mv = ln_stats(pool_stp, ps, R, eps_t)
                    ct = pool_tokp.tile([P, R], BF, tag="ckvtok")
                    nc.vector.tensor_scalar(out=ct, in0=ps,
                                            scalar1=mv[:, 0:1], scalar2=mv[:, 1:2],
                                            op0=ALU.subtract, op1=ALU.mult)
                    if kvln_wt is not None:
                        nc.vector.tensor_mul(ct, ct, kvln_wt)
                        nc.vector.tensor_add(ct, ct, kvln_bt)

                    evict_pairs(ro_q, qT, sl)

                    ckvTc = pool_ckvT.tile([P, NRC, P], BF, tag="ckvTc")
                    for rc in range(NRC):
                        pt = ps_tr.tile([P, P], BF, tag="ptr")
                        nc.tensor.transpose(pt, ct[:, rc * P:(rc + 1) * P], ident)
                        if rc % 2 == 0:
                            nc.scalar.copy(out=ckvTc[:, rc, :], in_=pt)
                        else:
                            nc.vector.tensor_copy(out=ckvTc[:, rc, :], in_=pt)

                    psk = ps_big.tile([P, HD], F32, tag="psb")
                    for rc in range(NRC):
                        nc.tensor.matmul(psk, lhsT=ckvTc[:, rc, :],
                                         rhs=kvuk_sb[:, rc, :],
                                         start=(rc == 0), stop=(rc == NRC - 1))
                    ro_k = rope(psk, cos_sb, sinf_sb)

                    psv = ps_big.tile([P, HD], F32, tag="psb")
                    for rc in range(NRC):
                        nc.tensor.matmul(psv, lhsT=ckvTc[:, rc, :],
                                         rhs=kvuv_sb[:, rc, :],
                                         start=(rc == 0), stop=(rc == NRC - 1))

                    evict_pairs(ro_k, kT, sl)

                    nc.vector.memset(vaug[:, it, :, D:D + 1], 1.0)
                    nc.scalar.copy(out=vaug[:, it, :, 0:D],
                                   in_=psv.rearrange("p (h d) -> p h d", d=D))

                stage_a(0)
                for it in range(NT):
                    if it + 1 < NT:
                        stage_a(it + 1)
                    stage_b(it)

            # ---- Phase 3+5a+5b: attention / proj / residual interleaved ---
            with ExitStack() as attn:
                pool_p = attn.enter_context(tc.tile_pool(name="pexp", bufs=16))
                pool_s = attn.enter_context(tc.tile_pool(name="srow", bufs=4))
                pool_bc = attn.enter_context(tc.tile_pool(name="bc", bufs=4))
                pool_pw = attn.enter_context(tc.tile_pool(name="pw", bufs=1))
                pool_sa = attn.enter_context(tc.tile_pool(name="sa", bufs=3))
                ps_s = attn.enter_context(
                    tc.tile_pool(name="pss", bufs=4, space="PSUM"))
                ps_o = attn.enter_context(
                    tc.tile_pool(name="pso", bufs=1, space="PSUM"))
                ps_pj = attn.enter_context(
                    tc.tile_pool(name="pspj", bufs=1, space="PSUM"))
                pw_sb = load_chunks(pool_pw, pw_d, NRC, C, "pw")
                NPAIR = NT // 2
                att_cur = [None]  # per-slab [(2h,64d), hp, SLB] tile

                def proj_slab(n):
                    # token-major partial: out[tok, C] = att[:, hp, tok].T @ pw
                    att = att_cur[0]
                    for itl in range(SLB // P):
                        tsl = slice(itl * P, (itl + 1) * P)
                        ps = ps_pj.tile([P, C], F32, tag="pspj")
                        for half in range(2):
                            hsl = slice(half * 512, (half + 1) * 512)
                            for hp in range(NRC):
                                nc.tensor.matmul(ps[:, hsl],
                                                 lhsT=att[:, hp, tsl],
                                                 rhs=pw_sb[:, hp, hsl],
                                                 start=(hp == 0),
                                                 stop=(hp == NRC - 1))
                        sa_t = pool_sa.tile([P, C], BF, tag="sat")
                        if projb_t is not None:
                            nc.vector.tensor_add(sa_t, ps, projb_t)
                        else:
                            nc.scalar.copy(out=sa_t, in_=ps)
                        nc.sync.dma_start(out=cc1_in[n][tsl, :], in_=sa_t)
                    nc.gpsimd.collective_compute(
                        "AllReduce", ALU.add, replica_groups=groups,
                        ins=[cc1_in[n]], outs=[cc1_out[n]])

                # Per head-pair, interleave the two heads (PE row groups 0-63
                # and 64-127) and pipeline qk of group g+1 ahead of av of
                # group g so the Exp never stalls the PE.
                GRP = 2  # key-blocks per score group
                for pi in range(NPAIR):
                    if pi % 2 == 0:
                        att_cur[0] = pool_att.tile([P, NRC, SLB], BF,
                                                   tag="attsl", name="attsl")
                    q0 = 2 * pi            # first q-block of pair
                    qsl = slice(q0 * P, (q0 + 2) * P)     # 256 queries
                    asl = slice((pi % 2) * 2 * P, (pi % 2 + 1) * 2 * P)
                    nkb = 2 * pi + 2
                    ngr = nkb // GRP

                    for hp in range(HL // 2):
                        poA = ps_o.tile([P, 2 * P], F32, tag="poA", name="poA")
                        poB = ps_o.tile([P, 2 * P], F32, tag="poB", name="poB")
                        pos = {0: poA, 64: poB}

                        def qk_group(g, hr):
                            pss = ps_s.tile([P, GRP * 2 * P], F32, tag="pss")
                            for j in range(GRP):
                                kb = g * GRP + j
                                jsl = slice(j * 2 * P, (j + 1) * 2 * P)
                                nc.tensor.matmul(
                                    pss[:, jsl],
                                    lhsT=kT[hr:hr + 64, hp, kb * P:(kb + 1) * P],
                                    rhs=qT[hr:hr + 64, hp, qsl],
                                    start=True, stop=True)
                                if kb == q0:
                                    nc.vector.tensor_add(
                                        pss[:, j * 2 * P:j * 2 * P + P],
                                        pss[:, j * 2 * P:j * 2 * P + P], maskT)
                                elif kb == q0 + 1:
                                    nc.vector.tensor_add(
                                        pss[:, jsl], pss[:, jsl],
                                        mask2.rearrange("p a b -> p (a b)"))
                            pexp = pool_p.tile([P, GRP * 2 * P], BF, tag="pexp")
                            nc.scalar.activation(out=pexp, in_=pss,
                                                 func=AF.Exp,
                                                 scale=float(D) ** -0.5)
                            return pexp

                        def av_group(g, hr, pexp):
                            po = pos[hr]
                            h = 2 * hp + (1 if hr else 0)
                            for j in range(GRP):
                                kb = g * GRP + j
                                nc.tensor.matmul(
                                    po[0:D + 1, :],
                                    lhsT=vaug[:, kb, h, :],
                                    rhs=pexp[:, j * 2 * P:(j + 1) * 2 * P],
                                    start=(kb == 0), stop=(kb == nkb - 1))

                        # all qk groups first (64-row array mode, the two
                        # heads in disjoint row groups), then all av groups
                        # (128-row mode): two mode switches per head-pair
                        # instead of two per group
                        pexps = []
                        for g in range(ngr):
                            pexps.append((qk_group(g, 0), qk_group(g, 64)))
                        for g in range(ngr):
                            av_group(g, 0, pexps[g][0])
                            av_group(g, 64, pexps[g][1])

                        # normalization fused into eviction
                        for hr in (0, 64):
                            po = pos[hr]
                            srow = pool_s.tile([1, 2 * P], F32, tag="srow")
                            nc.vector.tensor_copy(out=srow, in_=po[D:D + 1, :])
                            bc = pool_bc.tile([64, 2 * P], F32, tag="bc")
                            nc.gpsimd.partition_broadcast(out_ap=bc, in_ap=srow)
                            rc_ = pool_bc.tile([64, 2 * P], F32, tag="rc")
                            nc.vector.reciprocal_approx_fast(out=rc_, in_=bc)
                            nc.vector.tensor_mul(
                                att_cur[0][hr:hr + 64, hp, asl],
                                po[0:D, :], rc_)
                    if pi % 2 == 1:
                        proj_slab(pi // 2)

                # gate: depends on the last pair's attention eviction
                nc.vector.tensor_copy(out=gate_t,
                                      in_=att_cur[0][0:1, NRC - 1,
                                                     SLB - 4:SLB])

            mid.close()  # release qT/kT/vaug/att SBUF

            # ------- Back half: FFN + second AllReduce + final -------------
            with ExitStack() as bh:
                pool_fw = bh.enter_context(tc.tile_pool(name="fw", bufs=1))
                pool_g = bh.enter_context(tc.tile_pool(name="gT", bufs=2))
                pool_ev = bh.enter_context(tc.tile_pool(name="ev", bufs=3))
                pool_ffin = bh.enter_context(tc.tile_pool(name="ffin", bufs=3))
                pool_of = bh.enter_context(tc.tile_pool(name="of", bufs=3))
                ps_f = bh.enter_context(
                    tc.tile_pool(name="psf", bufs=2, space="PSUM"))
                ps_f2 = bh.enter_context(
                    tc.tile_pool(name="psf2", bufs=2, space="PSUM"))
                ps_tr1 = bh.enter_context(
                    tc.tile_pool(name="pstr1", bufs=2, space="PSUM"))
                fivb_pools["sa_in"] = bh.enter_context(
                    tc.tile_pool(name="sain", bufs=2))
                fivb_pools["x2"] = bh.enter_context(
                    tc.tile_pool(name="x2", bufs=2))
                fivb_pools["st2"] = bh.enter_context(
                    tc.tile_pool(name="st2", bufs=4))
                fivb_pools["tok2"] = bh.enter_context(
                    tc.tile_pool(name="tok2", bufs=4))

                # f1/f2 weight loads first: the DMAs overlap the attention
                # tail (fresh pool space, no released-zone wait on hot tiles)
                f1w_sb = load_chunks(pool_fw, f1_d, NCC, FH, "f1w")
                f2w_sb = load_chunks(pool_fw, f2_d, FH // P, C, "f2w")

                gT_sl = [None] * NSL
                NIT = SLB // P  # 4 token blocks per slab

                def f1_slab(n):
                    h2T = h2T_sl[n]
                    gT = pool_g.tile([P, FH // P, SLB], BF, tag="gT")
                    gT_sl[n] = gT
                    for m in range(FH // P):
                        ps = ps_f.tile([P, SLB], F32, tag="psf1")
                        for kc in range(NCC):
                            nc.tensor.matmul(ps,
                                             lhsT=f1w_sb[:, kc, m * P:(m + 1) * P],
                                             rhs=h2T[:, kc, :],
                                             start=(kc == 0),
                                             stop=(kc == NCC - 1))
                        if f1b_t is not None:
                            nc.scalar.activation(out=gT[:, m, :], in_=ps,
                                                 func=AF.Gelu,
                                                 bias=f1b_t[:, m:m + 1],
                                                 scale=1.0)
                        else:
                            nc.scalar.activation(out=gT[:, m, :], in_=ps,
                                                 func=AF.Gelu, scale=1.0)

                def cc2_home(it):
                    t = it * P
                    for ci, (t0, ln) in enumerate(CC2):
                        if t0 <= t < t0 + ln:
                            return ci, t - t0
                    raise AssertionError(it)

                def f2_block(it):
                    """f2 partial for global 128-token block it, token-major."""
                    n, itl = it // NIT, it % NIT
                    gT = gT_sl[n]
                    tsl = slice(itl * P, (itl + 1) * P)
                    ps = ps_f2.tile([P, C], F32, tag="psf2")
                    for half in range(2):
                        hsl = slice(half * 512, (half + 1) * 512)
                        for kf in range(FH // P):
                            nc.tensor.matmul(ps[:, hsl],
                                             lhsT=gT[:, kf, tsl],
                                             rhs=f2w_sb[:, kf, hsl],
                                             start=(kf == 0),
                                             stop=(kf == FH // P - 1))
                    ev = pool_ev.tile([P, C], BF, tag="ffev")
                    if f2b_t is not None:
                        nc.vector.tensor_add(ev, ps, f2b_t)
                    else:
                        nc.scalar.copy(out=ev, in_=ps)
                    ci, row = cc2_home(it)
                    nc.sync.dma_start(out=cc2_in[ci][row:row + P, :], in_=ev)

                def cc2_launch(ci):
                    nc.gpsimd.collective_compute(
                        "AllReduce", ALU.add, replica_groups=groups,
                        ins=[cc2_in[ci]], outs=[cc2_out[ci]])

                def emit_final(ci):
                    """out = x + sa + ff for collective chunk ci (no PE)."""
                    t0, ln = CC2[ci]
                    for j in range(ln // P):
                        itg = t0 // P + j
                        sl = slice(itg * P, (itg + 1) * P)
                        xt = pool_xs.tile([P, C], F32, tag="xt2")
                        nc.sync.dma_start(out=xt, in_=x_d[sl, :])
                        saf = pool_ffin.tile([P, C], BF, tag="sfin")
                        nc.vector.tensor_copy(out=saf[0:1, 0:4], in_=gate_t)
                        nc.sync.dma_start(
                            out=saf,
                            in_=cc1_out[itg // NIT][(itg % NIT) * P:
                                                    (itg % NIT + 1) * P, :])
                        ff = pool_ffin.tile([P, C], BF, tag="ffin")
                        nc.vector.tensor_copy(out=ff[0:1, 0:4], in_=gate_t)
                        nc.sync.dma_start(out=ff,
                                          in_=cc2_out[ci][j * P:(j + 1) * P, :])
                        ot = pool_of.tile([P, C], F32, tag="of")
                        nc.vector.tensor_add(ot, xt, saf)
                        nc.gpsimd.tensor_add(ot, ot, ff)
                        nc.sync.dma_start(out=out_d[sl, :], in_=ot)

                emit_5b(0)
                f1_slab(0)
                emit_5b(1)
                for it in range(0, 4):
                    f2_block(it)
                cc2_launch(0)
                f1_slab(1)
                emit_5b(2)
                for it in range(4, 8):
                    f2_block(it)
                cc2_launch(1)
                emit_final(0)
                f1_slab(2)
                emit_5b(3)
                for it in range(8, 12):
                    f2_block(it)
                cc2_launch(2)
                emit_final(1)
                f1_slab(3)
                f2_block(12)
                f2_block(13)
                f2_block(14)
                cc2_launch(3)
                emit_final(2)
                f2_block(15)
                cc2_launch(4)
                emit_final(3)
                emit_final(4)

    nc.compile()
    return nc


def kernel(**inputs):
    x = np.asarray(inputs["x"], dtype=np.float32)
    q_w = np.asarray(inputs["q_w"], dtype=np.float32)
    kvd_w = np.asarray(inputs["kvd_w"], dtype=np.float32)
    kvu_w = np.asarray(inputs["kvu_w"], dtype=np.float32)
    proj_w = np.asarray(inputs["proj_w"], dtype=np.float32)
    f1_w = np.asarray(inputs["f1_w"], dtype=np.float32)
    f2_w = np.asarray(inputs["f2_w"], dtype=np.float32)
    ln1_w = np.asarray(inputs["ln1_w"], dtype=np.float32)
    ln1_b = np.asarray(inputs["ln1_b"], dtype=np.float32)
    ln2_w = np.asarray(inputs["ln2_w"], dtype=np.float32)
    ln2_b = np.asarray(inputs["ln2_b"], dtype=np.float32)
    kvln_w = np.asarray(inputs["kvln_w"], dtype=np.float32)
    kvln_b = np.asarray(inputs["kvln_b"], dtype=np.float32)
    proj_b = np.asarray(inputs["proj_b"], dtype=np.float32)
    f1_b = np.asarray(inputs["f1_b"], dtype=np.float32)
    f2_b = np.asarray(inputs["f2_b"], dtype=np.float32)

    flags = (bool(np.allclose(ln1_w, 1) and np.allclose(ln1_b, 0)),
             bool(np.allclose(kvln_w, 1) and np.allclose(kvln_b, 0)),
             bool(np.allclose(ln2_w, 1) and np.allclose(ln2_b, 0)),
             bool(np.allclose(proj_b, 0)),
             bool(np.allclose(f1_b, 0)),
             bool(np.allclose(f2_b, 0)))
    if flags not in _CACHE:
        _CACHE[flags] = _build(flags)
    nc = _CACHE[flags]

    cos, sinf = _rope_tables()
    kvu_v4 = kvu_w.reshape(R, 2, H, D)
    in_maps = []
    for c in range(8):
        b, half = c // 2, c % 2
        hsl = slice(half * HL, (half + 1) * HL)
        m = {
            "x_loc": np.ascontiguousarray(x[b]),
            "qw_loc": np.ascontiguousarray(
                q_w[:, half * HD:(half + 1) * HD]).astype(BF16),
            "kvd_w": kvd_w.astype(BF16),
            "kvu_k": np.ascontiguousarray(
                kvu_v4[:, 0, hsl, :].reshape(R, HD)).astype(BF16),
            "kvu_v": np.ascontiguousarray(
                kvu_v4[:, 1, hsl, :].reshape(R, HD)).astype(BF16),
            "proj_w_loc": np.ascontiguousarray(
                proj_w[half * HD:(half + 1) * HD, :]).astype(BF16),
            "f1_w_loc": np.ascontiguousarray(
                f1_w[:, half * FH:(half + 1) * FH]).astype(BF16),
            "f2_w_loc": np.ascontiguousarray(
                f2_w[half * FH:(half + 1) * FH, :]).astype(BF16),
            "cos_t": cos.astype(BF16),
            "sinf_t": sinf.astype(BF16),
        }
        if not flags[0]:
            m["ln1_w"], m["ln1_b"] = ln1_w, ln1_b
        if not flags[1]:
            m["kvln_w"], m["kvln_b"] = kvln_w, kvln_b
        if not flags[2]:
            m["ln2_w"], m["ln2_b"] = ln2_w, ln2_b
        if not flags[3]:
            m["proj_b"] = proj_b
        if not flags[4]:
            m["f1_b_loc"] = np.ascontiguousarray(f1_b[half * FH:(half + 1) * FH])
        if not flags[5]:
            m["f2_b"] = f2_b
        in_maps.append(m)

    res = run_bass_kernel_spmd(nc, in_maps, list(range(8)), trace=TRACE)
    kernel.last_result = res
    out = np.stack([res.results[2 * b]["out_loc"] for b in range(B)])
    return out

